# revision 1
# baseline (speedup 1.0000x reference)
"""Trainium2 Bass kernel: modulated deformable conv 3x3 (DCNv2) + BN(eval)
+ ReLU.  B=4, C=O=256, H=W=64, distributed over 8 NeuronCores.

Sharding: core i -> batch b = i//2, image row-half = i%2 (32 rows). Each core
computes out[b, :, h0:h0+32, :] fully.

Per-core pipeline (all x-dependent compute on device):
  - xpad [C,74*74] fp16 zero-padded image (pad P=5) via casting SWDGE DMA.
  - offset conv om[27,2048] from a 34-row halo slice (9 shifted matmuls).
  - om -> omT [hw_part, 27]; DVE computes bilinear corner weights (kept as
    per-partition scalars) and int16 gather indices (clamped to pad; pad is
    zero so out-of-image corners contribute 0, matching the reference).
  - XT1 = xpad^T in HBM scratch [5504, 256] fp16 (PE transposes).
  - per (s-block, tap): dma_gather of 4 corners x 1024 samples (elem 512B)
    into [sample, c] layout; DVE combines corners via tensor_scalar + 3
    fused scalar_tensor_tensor (per-partition weights); PE transposes
    V^T -> V[c,s]; matmuls accumulate out^T[s,o] over (tap,cc) in PSUM;
    bias via K=1 matmul; ReLU; PE transpose to [o,s]; store.
BN is folded on device: W' = W * (gamma*rsqrt(var+eps)) ; b' = s*(bias-mean)+beta.
"""

import numpy as np

import concourse.bass as bass
import concourse.bacc as bacc
import concourse.mybir as mybir
import concourse.tile as tile
from concourse import bass_utils, library_config

F32 = mybir.dt.float32
F16 = mybir.dt.float16
I16 = mybir.dt.int16
I32 = mybir.dt.int32
AF = mybir.ActivationFunctionType
ALU = mybir.AluOpType

B, C, O, H, W = 4, 256, 256, 64, 64
K = 9
P = 5
W2 = H + 2 * P            # 74
NQ = W2 * W2              # 5476
NQP = 5504                # 43*128
HH = 32                   # rows per core
S = HH * W                # 2048 samples per core
CC = C // 128             # 2
OCC = O // 128            # 2
NSB = 2                   # sample blocks
SB = S // NSB             # 1024 samples / block
CHB = 8                   # chunks per block
NI = 4 * SB               # idxs per gather call (4 corners) = 4096
EPS = 1e-5
N_CORES = 8

_NC_CACHE = {}


def build_nc(debug_om=False, stop_after=None):
    nc = bacc.Bacc("TRN2", target_bir_lowering=False, debug=False,
                   num_devices=N_CORES)

    x_in = nc.dram_tensor("x_b", [C, H, W], F32, kind="ExternalInput")
    xhalo = nc.dram_tensor("xhalo", [C, 34, W], F32, kind="ExternalInput")
    w_t = nc.dram_tensor("w_t", [CC, 128, K * O], F32, kind="ExternalInput")
    woff_t = nc.dram_tensor("woff_t", [CC, 128, K * 27], F32, kind="ExternalInput")
    b_off_in = nc.dram_tensor("b_off", [27, 1], F32, kind="ExternalInput")
    bnvec = nc.dram_tensor("bnvec", [1, 5 * O], F32, kind="ExternalInput")
    baseC = nc.dram_tensor("baseC", [128, 16 * 32], F32, kind="ExternalInput")
    ident32 = nc.dram_tensor("ident32", [128, 128], F32, kind="ExternalInput")
    ident16 = nc.dram_tensor("ident16", [128, 128], F16, kind="ExternalInput")
    ones16 = nc.dram_tensor("ones16", [1, 128], F16, kind="ExternalInput")

    out_d = nc.dram_tensor("out_c", [O, S], F32, kind="ExternalOutput")
    om_dbg = (nc.dram_tensor("om_dbg", [27, S], F32, kind="ExternalOutput")
              if (debug_om or stop_after) else None)
    dbg = {}
    if stop_after in ("idx", "xt1", "gather"):
        dbg["idx"] = nc.dram_tensor("dbg_idx", [128, 256], I16, kind="ExternalOutput")
    if stop_after in ("xt1", "gather"):
        dbg["xt"] = nc.dram_tensor("dbg_xt", [128, 256], F16, kind="ExternalOutput")
    if stop_after == "gather":
        dbg["g"] = nc.dram_tensor("dbg_g", [128, 32, 256], F16, kind="ExternalOutput")

    with tile.TileContext(nc) as tc:
        _build(nc, tc, x_in, xhalo, w_t, woff_t, b_off_in, bnvec, baseC,
               ident32, ident16, ones16, out_d, om_dbg, stop_after, dbg if stop_after else {})
    nc.compile()
    return nc


def _build(nc, tc, x_in, xhalo, w_t, woff_t, b_off_in, bnvec, baseC,
           ident32, ident16, ones16, out_d, om_dbg, stop_after=None, dbg=None):
    from contextlib import ExitStack

    with ExitStack() as top:
        pers = top.enter_context(tc.tile_pool(name="pers", bufs=1))
        dram = top.enter_context(tc.tile_pool(name="dram", bufs=1, space="DRAM"))
        xt1_t = dram.tile([NQP, C], F16, name="xt1_scr", tag="xt1")
        idxs_t = dram.tile([K, NSB, 16, 256], I16, name="idx_scr", tag="idxs")
        xt1 = xt1_t.tensor if hasattr(xt1_t, "tensor") else None
        # use AP-level access below
        ph1_cm = tc.tile_pool(name="ph1", bufs=1)
        ph1 = ph1_cm.__enter__()

        # ------------- constants -------------
        id32 = pers.tile([128, 128], F32)
        nc.sync.dma_start(out=id32[:], in_=ident32.ap())
        id16 = pers.tile([128, 128], F16)
        nc.sync.dma_start(out=id16[:], in_=ident16.ap())
        one16 = pers.tile([1, 128], F16)
        nc.sync.dma_start(out=one16[:], in_=ones16.ap())
        base_t = pers.tile([128, 16, 32], F32)
        nc.sync.dma_start(out=base_t[:], in_=baseC.ap().rearrange("p (a b) -> p a b", a=16))
        boff_t = pers.tile([27, 1], F32)
        nc.sync.dma_start(out=boff_t[:], in_=b_off_in.ap())

        # ------------- BN fold -------------
        bn_t = pers.tile([1, 5 * O], F32)
        nc.sync.dma_start(out=bn_t[:], in_=bnvec.ap())
        gam = bn_t[:, 0:O]; bet = bn_t[:, O:2 * O]; rmn = bn_t[:, 2 * O:3 * O]
        rvr = bn_t[:, 3 * O:4 * O]; bia = bn_t[:, 4 * O:5 * O]
        sq = pers.tile([1, O], F32)
        nc.vector.tensor_scalar(sq[:], rvr, float(EPS), None, ALU.add)
        nc.scalar.activation(sq[:], sq[:], AF.Sqrt)
        sfac = pers.tile([1, O], F32)
        nc.vector.reciprocal(sfac[:], sq[:])
        nc.vector.tensor_tensor(out=sfac[:], in0=sfac[:], in1=gam, op=ALU.mult)
        bpr = pers.tile([1, O], F32)
        nc.vector.tensor_tensor(out=bpr[:], in0=bia, in1=rmn, op=ALU.subtract)
        nc.vector.tensor_tensor(out=bpr[:], in0=bpr[:], in1=sfac[:], op=ALU.mult)
        nc.vector.tensor_tensor(out=bpr[:], in0=bpr[:], in1=bet, op=ALU.add)
        bprow16 = pers.tile([1, O], F16)
        nc.vector.tensor_copy(bprow16[:], bpr[:])
        sbc = pers.tile([128, O], F32)
        ones32 = pers.tile([1, 128], F32)
        nc.vector.memset(ones32[:], 1.0)
        with tc.tile_pool(name="bcp", bufs=1, space="PSUM") as bcp:
            bc_ps = bcp.tile([128, O], F32)
            nc.tensor.matmul(bc_ps[:], ones32[:], sfac[:], start=True, stop=True)
            nc.vector.tensor_copy(sbc[:], bc_ps[:])

        # ------------- weights -------------
        wmain = []
        for cc in range(CC):
            wmain.append(pers.tile([128, K * O], F16, name=f"wmain{cc}", tag=f"wmain{cc}"))
        woff16 = []
        for cc in range(CC):
            woff16.append(ph1.tile([128, K * 27], F16, name=f"woff{cc}", tag=f"woff{cc}"))

        with tc.tile_pool(name="wtmp", bufs=1) as wtmp:
            for cc in range(CC):
                wr = wtmp.tile([128, K * O], F32, name=f"wr{cc}", tag="wr")
                nc.sync.dma_start(out=wr[:], in_=w_t.ap()[cc])
                for k in range(K):
                    nc.vector.tensor_tensor(out=wmain[cc][:, k * O:(k + 1) * O],
                                            in0=wr[:, k * O:(k + 1) * O],
                                            in1=sbc[:], op=ALU.mult)
                wo = wtmp.tile([128, K * 27], F32, name=f"wo{cc}", tag="wo")
                nc.sync.dma_start(out=wo[:], in_=woff_t.ap()[cc])
                nc.vector.tensor_copy(woff16[cc][:], wo[:])

        # ------------- xpad (full, fp16) + xom (halo, fp16) -------------
        xpad = []
        for cc in range(CC):
            t = ph1.tile([128, NQP], F16, name=f"xpad{cc}", tag=f"xpad{cc}")
            nc.vector.memset(t[:], 0.0)
            dst = t[:, 0:NQ].rearrange("p (h w) -> p h w", w=W2)[:, P:P + H, P:P + W]
            nc.gpsimd.dma_start(out=dst, in_=x_in.ap()[cc * 128:(cc + 1) * 128])
            xpad.append(t)
        xom = []
        for cc in range(CC):
            t = ph1.tile([128, 34 * W2], F16, name=f"xom{cc}", tag=f"xom{cc}")
            nc.vector.memset(t[:], 0.0)
            dst = t[:].rearrange("p (h w) -> p h w", w=W2)[:, :, P:P + W]
            nc.gpsimd.dma_start(out=dst, in_=xhalo.ap()[cc * 128:(cc + 1) * 128])
            xom.append(t)

        # ------------- offset conv: om [27, 2048] -------------
        om_sb = ph1.tile([27, S], F32)
        omT = ph1.tile([128, 16, 32], F32)
        with tc.tile_pool(name="omps", bufs=1, space="PSUM") as omps:
            om_ps = omps.tile([27, S], F32, name="om_ps", tag="om_ps")
            for bk in range(4):           # 4 banks of 512 (8 rows x 64)
                for cc in range(CC):
                    for t9 in range(K):
                        ty, tx = t9 // 3, t9 % 3
                        rhs = xom[cc][:].rearrange("p (h w) -> p h w", w=W2)[
                            :, bk * 8 + ty: bk * 8 + ty + 8,
                            P - 1 + tx: P - 1 + tx + W]
                        nc.tensor.matmul(om_ps[:, bk * 512:(bk + 1) * 512],
                                         woff16[cc][:, t9 * 27:(t9 + 1) * 27], rhs,
                                         start=(cc == 0 and t9 == 0),
                                         stop=(cc == CC - 1 and t9 == K - 1))
            nc.scalar.activation(om_sb[:], om_ps[:], AF.Identity,
                                 bias=boff_t[:, 0:1])
            if om_dbg is not None:
                nc.sync.dma_start(out=om_dbg.ap(), in_=om_sb[:])

            # ------------- omT [128, 16, 32] -------------
            omT_ps = omps.tile([128, 512], F32, name="omT_ps", tag="omT_ps")
            nc.vector.memset(omT_ps[:], 0.0)
            for ch in range(16):
                nc.tensor.transpose(omT_ps[:, ch * 32: ch * 32 + 27],
                                    om_sb[:, ch * 128:(ch + 1) * 128],
                                    id32[0:27, 0:27])
            nc.vector.tensor_copy(omT[:],
                                  omT_ps[:].rearrange("p (a b) -> p a b", a=16))
        if stop_after == "om":
            ph1_cm.__exit__(None, None, None)
            return

        # ------------- sample math -------------
        ppx = ph1.tile([128, 16, 32], F32)
        nc.vector.tensor_tensor(out=ppx[:], in0=omT[:], in1=base_t[:], op=ALU.add)
        ii = ph1.tile([128, 16, 18], I32)
        nc.vector.tensor_copy(ii[:], ppx[:, :, 0:18])
        ff = ph1.tile([128, 16, 18], F32)
        nc.vector.tensor_copy(ff[:], ii[:])
        gtt = ph1.tile([128, 16, 18], F32)
        nc.vector.tensor_tensor(out=gtt[:], in0=ff[:], in1=ppx[:, :, 0:18], op=ALU.is_gt)
        flo = ph1.tile([128, 16, 18], F32)
        nc.vector.tensor_tensor(out=flo[:], in0=ff[:], in1=gtt[:], op=ALU.subtract)
        lf = ph1.tile([128, 16, 18], F32)
        nc.vector.tensor_tensor(out=lf[:], in0=ppx[:, :, 0:18], in1=flo[:], op=ALU.subtract)
        floc = ph1.tile([128, 16, 18], F32)
        nc.vector.tensor_scalar(floc[:], flo[:], 0.0, float(W2 - 2), ALU.max, ALU.min)
        msk = ph1.tile([128, 16, 9], F32)
        nc.scalar.activation(msk[:], omT[:, :, 18:27], AF.Sigmoid)
        ol = ph1.tile([128, 16, 18], F32)
        nc.vector.tensor_scalar(ol[:], lf[:], -1.0, 1.0, ALU.mult, ALU.add)
        # corner weights (with mask folded): [128, 16, 9] each
        wr4 = []
        for r, (ya, xa) in enumerate([(0, 0), (0, 1), (1, 0), (1, 1)]):
            yw = ol if ya == 0 else lf     # (1-ly) or ly
            xw = ol if xa == 0 else lf
            wtile = pers.tile([128, 16, 9], F32, name=f"wr4_{r}", tag=f"wr4_{r}")
            nc.vector.tensor_tensor(out=wtile[:], in0=yw[:, :, 0:9],
                                    in1=xw[:, :, 9:18], op=ALU.mult)
            nc.vector.tensor_tensor(out=wtile[:], in0=wtile[:], in1=msk[:], op=ALU.mult)
            wr4.append(wtile)
        # gather index base q = y0c*W2 + x0c
        qf = ph1.tile([128, 16, 9], F32)
        nc.vector.tensor_scalar(qf[:], floc[:, :, 0:9], float(W2), None, ALU.mult)
        nc.vector.tensor_tensor(out=qf[:], in0=qf[:], in1=floc[:, :, 9:18], op=ALU.add)

        # staging per tap: [128, 16ch, 4r] f32, then transpose+permute+cast
        idxT16 = pers.tile([64, K * 128], I16)
        with tc.tile_pool(name="idxp", bufs=2) as idxp, \
             tc.tile_pool(name="idxps", bufs=3, space="PSUM") as idxps:
            for k in range(K):
                stg = idxp.tile([128, 16, 4], F32, name=f"stg{k}", tag="stg")
                for r, dr in enumerate([0, 1, W2, W2 + 1]):
                    nc.vector.tensor_scalar(stg[:, :, r:r + 1], qf[:, :, k:k + 1],
                                            float(dr), None, ALU.add)
                tps = idxps.tile([64, 128], F32, name=f"tps{k}", tag="tps")
                nc.tensor.transpose(tps[:], stg[:].rearrange("p a b -> p (a b)"),
                                    id32[:])
                # permuting cast copy: out[., q*8+jj] = in[., jj*16+q]
                src = tps[:].rearrange("p (jj q) -> p q jj", jj=8)
                dst = idxT16[:, k * 128:(k + 1) * 128].rearrange(
                    "p (q jj) -> p q jj", q=16)
                nc.vector.tensor_copy(dst, src)

        # hop1: SBUF -> DRAM wrapped layout; hop2: DRAM -> SBUF + replicate
        wrapped = []
        for k in range(K):
            for sblk in range(NSB):
                # dst dims (ch' 8, r 4 | q 16, jj 8) ; src [32 part, q, jj]
                dstd = idxs_t[k, sblk].rearrange(
                    "q (chp r jj) -> q chp r jj", chp=8, r=4)
                dstd = dstd.transpose([1, 2, 0, 3])   # (chp, r, q, jj)
                src = idxT16[sblk * 32:(sblk + 1) * 32,
                             k * 128:(k + 1) * 128].rearrange(
                    "p (q jj) -> p q jj", q=16)
                nc.sync.dma_start(out=dstd, in_=src)
        for k in range(K):
            for sblk in range(NSB):
                wt_ = pers.tile([128, 256], I16, name=f"wrp{k}_{sblk}",
                                tag=f"wrp{k}_{sblk}")
                for g in range(8):
                    nc.sync.dma_start(out=wt_[g * 16:(g + 1) * 16, :],
                                      in_=idxs_t[k, sblk])
                wrapped.append(wt_)

        if stop_after == "idx":
            nc.sync.dma_start(out=dbg["idx"].ap(), in_=wrapped[0][:])
            ph1_cm.__exit__(None, None, None)
            return

        # ------------- XT1 build -------------
        with tc.tile_pool(name="xtp", bufs=2) as xtp, \
             tc.tile_pool(name="xtps", bufs=2, space="PSUM") as xtps:
            for grp in range(11):          # 4 q-chunks per group, 43 chunks
                qcs = range(grp * 4, min(grp * 4 + 4, 43))
                pt = xtps.tile([128, 1024], F16, name=f"xt_ps{grp}", tag="xt_ps")
                for i, qc in enumerate(qcs):
                    for cc in range(CC):
                        nc.tensor.transpose(
                            pt[:, i * 256 + cc * 128: i * 256 + (cc + 1) * 128],
                            xpad[cc][:, qc * 128:(qc + 1) * 128], id16[:])
                st = xtp.tile([128, 1024], F16, name=f"xt_sb{grp}", tag="xt_sb")
                nqc = len(list(qcs))
                nc.scalar.activation(st[:, 0:nqc * 256], pt[:, 0:nqc * 256], AF.Copy)
                dst = xt1_t[grp * 512: grp * 512 + nqc * 128, :].rearrange(
                    "(qc p) c -> p qc c", p=128)
                nc.sync.dma_start(out=dst,
                                  in_=st[:, 0:nqc * 256].rearrange(
                                      "p (qc c) -> p qc c", c=256))

        if stop_after == "xt1":
            nc.sync.dma_start(out=dbg["idx"].ap(), in_=wrapped[0][:])
            xtrd = pers.tile([128, 256], F16, name="xtrd", tag="xtrd")
            nc.sync.dma_start(out=xtrd[:], in_=xt1_t[0:128, :])
            nc.sync.dma_start(out=dbg["xt"].ap(), in_=xtrd[:])
            ph1_cm.__exit__(None, None, None)
            return
        if stop_after == "gather":
            gt0 = pers.tile([128, 32, 256], F16, name="gt0", tag="gt0")
            nc.gpsimd.dma_gather(gt0[:], xt1_t[:, :], wrapped[0][:], NI, NI, 256, single_packet=False)
            nc.sync.dma_start(out=dbg["g"].ap(), in_=gt0[:])
            nc.sync.dma_start(out=dbg["idx"].ap(), in_=wrapped[0][:])
            xtrd = pers.tile([128, 256], F16, name="xtrd", tag="xtrd")
            nc.sync.dma_start(out=xtrd[:], in_=xt1_t[0:128, :])
            nc.sync.dma_start(out=dbg["xt"].ap(), in_=xtrd[:])
            ph1_cm.__exit__(None, None, None)
            return

        # ------------- main loop -------------
        ph1_cm.__exit__(None, None, None)
        out_osb = [pers.tile([128, S], F32, name=f"out_osb{occ}", tag=f"oo{occ}")
                   for occ in range(OCC)]

        with tc.tile_pool(name="mg", bufs=2) as mg, \
             tc.tile_pool(name="mv", bufs=2) as mv, \
             tc.tile_pool(name="mvs", bufs=1) as mvs, \
             tc.tile_pool(name="mps", bufs=1, space="PSUM") as mps, \
             tc.tile_pool(name="accp", bufs=3, space="PSUM") as accp, \
             tc.tile_pool(name="outp", bufs=2) as outp, \
             tc.tile_pool(name="outps", bufs=1, space="PSUM") as outps:
            for sblk in range(NSB):
                vsb = [[None] * CC for _ in range(K)]
                for k in range(K):
                    gt = mg.tile([128, 32, 256], F16, name=f"g{sblk}_{k}", tag="gt")
                    nc.gpsimd.dma_gather(gt[:], xt1_t[:, :], wrapped[k * NSB + sblk][:],
                                         NI, NI, 256, single_packet=False)
                    vt = mv.tile([128, CHB * 256], F16, name=f"v{sblk}_{k}", tag="vt")
                    for chp in range(CHB):
                        ch = sblk * CHB + chp
                        vts = vt[:, chp * 256:(chp + 1) * 256]
                        nc.vector.tensor_scalar(
                            vts, gt[:, chp * 4 + 0, :],
                            wr4[0][:, ch, k:k + 1], None, ALU.mult)
                        for r in range(1, 4):
                            nc.vector.scalar_tensor_tensor(
                                out=vts, in0=gt[:, chp * 4 + r, :],
                                scalar=wr4[r][:, ch, k:k + 1], in1=vts,
                                op0=ALU.mult, op1=ALU.add)
                    # transpose V^T -> V [c, s]
                    vps = [mps.tile([128, 1024], F16, name=f"vps{sblk}_{k}_{cc}",
                                    tag=f"vps{cc}") for cc in range(CC)]
                    for chp in range(CHB):
                        for cc in range(CC):
                            nc.tensor.transpose(
                                vps[cc][:, chp * 128:(chp + 1) * 128],
                                vt[:, chp * 256 + cc * 128: chp * 256 + (cc + 1) * 128],
                                id16[:])
                    for cc in range(CC):
                        t = mvs.tile([128, 1024], F16, name=f"vsb{sblk}_{k}_{cc}",
                                     tag=f"vsb{k}_{cc}")
                        nc.scalar.activation(t[:], vps[cc][:], AF.Copy)
                        vsb[k][cc] = t
                # matmuls: per s-chunk, own PSUM bank, accumulate over (k, cc)
                for chp in range(CHB):
                    ch = sblk * CHB + chp
                    acc = accp.tile([128, O], F32, name=f"acc{sblk}_{chp}",
                                    tag="acc")
                    for k in range(K):
                        for cc in range(CC):
                            nc.tensor.matmul(
                                acc[:],
                                vsb[k][cc][:, chp * 128:(chp + 1) * 128],
                                wmain[cc][:, k * O:(k + 1) * O],
                                start=(k == 0 and cc == 0), stop=False)
                    nc.tensor.matmul(acc[:], one16[:], bprow16[:],
                                     start=False, stop=True)
                    relu = outp.tile([128, O], F32, name=f"relu{sblk}_{chp}",
                                     tag="relu")
                    nc.scalar.activation(relu[:], acc[:], AF.Relu)
                    ops_ = outps.tile([128, 256], F32, name=f"ops{sblk}_{chp}",
                                      tag="ops")
                    for occ in range(OCC):
                        nc.tensor.transpose(
                            ops_[:, occ * 128:(occ + 1) * 128],
                            relu[:, occ * 128:(occ + 1) * 128],
                            id32[:])
                    for occ in range(OCC):
                        nc.vector.tensor_copy(
                            out_osb[occ][:, ch * 128:(ch + 1) * 128],
                            ops_[:, occ * 128:(occ + 1) * 128])
        for occ in range(OCC):
            nc.sync.dma_start(out=out_d.ap()[occ * 128:(occ + 1) * 128, :],
                              in_=out_osb[occ][:])


# ===================== host side =====================

def _host_prep(inputs):
    """Build the 8 per-core input maps (layout-only host work + constants)."""
    x = np.ascontiguousarray(inputs["x"], dtype=np.float32)
    w_off = np.asarray(inputs["w_off"], np.float32)
    b_off = np.asarray(inputs["b_off"], np.float32)
    weight = np.asarray(inputs["weight"], np.float32)
    bias = np.asarray(inputs["bias"], np.float32)
    gamma = np.asarray(inputs["gamma"], np.float32)
    beta = np.asarray(inputs["beta"], np.float32)
    run_mean = np.asarray(inputs["run_mean"], np.float32)
    run_var = np.asarray(inputs["run_var"], np.float32)

    # weight [O, C, 3, 3] -> [CC, 128c, K, O] -> [CC, 128, K*O]
    wt = weight.reshape(O, C, K).transpose(1, 2, 0).reshape(CC, 128, K * O)
    wt = np.ascontiguousarray(wt)
    # w_off [27, C, 3, 3] -> [CC, 128c, K, 27]
    wofft = w_off.reshape(27, C, K).transpose(1, 2, 0).reshape(CC, 128, K * 27)
    wofft = np.ascontiguousarray(wofft)
    bnv = np.concatenate([gamma, beta, run_mean, run_var, bias]).astype(np.float32).reshape(1, 5 * O)
    id32 = np.eye(128, dtype=np.float32)
    id16 = np.eye(128, dtype=np.float16)
    ones = np.ones((1, 128), np.float16)
    boff = b_off.reshape(27, 1).astype(np.float32)

    in_maps = []
    for core in range(N_CORES):
        b, half = core // 2, core % 2
        h0 = half * HH
        # halo rows [h0-1, h0+33) with zero pad at the image boundary
        halo = np.zeros((C, 34, W), np.float32)
        lo, hi = h0 - 1, h0 + 33
        slo, shi = max(lo, 0), min(hi, H)
        halo[:, slo - lo: slo - lo + (shi - slo)] = x[b, :, slo:shi]
        # baseC [128, 16, 32]: cols 0-8 pyP base, 9-17 pxP base, rest 0
        basec = np.zeros((128, 16, 32), np.float32)
        pp_ = np.arange(128)
        for ch in range(16):
            s_ = ch * 128 + pp_
            hloc = h0 + s_ // W
            wloc = s_ % W
            for k in range(K):
                basec[:, ch, k] = hloc + (k // 3) - 1 + P
                basec[:, ch, 9 + k] = wloc + (k % 3) - 1 + P
        in_maps.append({
            "x_b": np.ascontiguousarray(x[b]),
            "xhalo": halo,
            "w_t": wt,
            "woff_t": wofft,
            "b_off": boff,
            "bnvec": bnv,
            "baseC": basec.reshape(128, 16 * 32),
            "ident32": id32,
            "ident16": id16,
            "ones16": ones,
        })
    return in_maps


def _get_nc():
    if "nc" not in _NC_CACHE:
        _NC_CACHE["nc"] = build_nc()
    return _NC_CACHE["nc"]


def kernel(**inputs):
    nc = _get_nc()
    in_maps = _host_prep(inputs)
    res = bass_utils.run_bass_kernel_spmd(nc, in_maps, core_ids=list(range(N_CORES)))
    out = np.zeros((B, O, H, W), np.float32)
    for core in range(N_CORES):
        b, half = core // 2, core % 2
        out[b, :, half * HH:(half + 1) * HH, :] = (
            res.results[core]["out_c"].reshape(O, HH, W))
    return out



# revision 4
# speedup vs baseline: 2.2762x; 2.2762x over previous
"""Trainium2 Bass kernel: modulated deformable conv 3x3 (DCNv2) + BN(eval)
+ ReLU.  B=4, C=O=256, H=W=64, distributed over 8 NeuronCores.

Sharding: core i -> batch b = i//2, image row-half = i%2 (32 rows). Each core
computes out[b, :, h0:h0+32, :] fully.

v2 design (quad-token gather):
  - xpad [C,74*74] fp16 zero-padded image (pad P=5) via casting SWDGE DMA;
    xom [C,34*74] halo rows for the offset conv (host-sliced per core).
  - offset conv om[27,2048] (9 shifted matmuls x 2 cc, 4 PSUM banks).
  - omT -> bilinear corner weights wr4 (kept as per-partition scalars) and
    ONE int16 token index per (sample, tap): q = (y0+1)*74 + x0 (clamped to
    the zero pad, so out-of-image corners contribute 0).
  - XT-quad DRAM scratch [NTOK,512] fp16: row r = [X^T[r-74] | X^T[r]].  A
    single 2048B descriptor starting at row r covers rows r,r+1 = all four
    bilinear corners of a sample: 1 descriptor per (sample, tap) instead of
    4, quartering SWDGE descriptor-generation time (the v1 bottleneck).
  - per (sblk, k): one dma_gather (NI=1024, elem 2048B, elem_step 1024B) ->
    gt [128s, 8chp, 1024]; DVE combines the 4 corners (tensor_scalar + 3
    fused scalar_tensor_tensor with per-partition weights); PE transposes
    V^T -> V[c,s]; weight-stationary matmuls accumulate out^T[o,s] in PSUM
    over (k, cc); Scalar engine applies folded BN bias + ReLU from PSUM;
    DMA out^T[o,s] fp32.
BN + conv bias are folded on host: W' = W * (gamma*rsqrt(var+eps)) ;
b' = s*(bias-mean)+beta, applied as per-partition activation bias.
"""

import numpy as np

import concourse.bass as bass
import concourse.bacc as bacc
import concourse.mybir as mybir
import concourse.tile as tile
from concourse import bass_utils, library_config

F32 = mybir.dt.float32
F16 = mybir.dt.float16
I16 = mybir.dt.int16
I32 = mybir.dt.int32
AF = mybir.ActivationFunctionType
ALU = mybir.AluOpType

B, C, O, H, W = 4, 256, 256, 64, 64
K = 9
P = 5
W2 = H + 2 * P            # 74
NQ = W2 * W2              # 5476
NQP = 5504                # 43*128 (transposed q count, padded)
NTOK = 74 + NQP           # 5578 quad rows (front pad of 74)
HH = 32                   # rows per core
S = HH * W                # 2048 samples per core
CC = C // 128             # 2
OCC = O // 128            # 2
NSB = 2                   # sample blocks
SB = S // NSB             # 1024 samples / block
CHB = 8                   # 128-sample chunks per block
NI = SB                   # gather indices per call (1 per sample)
EPS = 1e-5
N_CORES = 8

_NC_CACHE = {}


def build_nc():
    nc = bacc.Bacc("TRN2", target_bir_lowering=False, debug=False,
                   num_devices=N_CORES)

    x_in = nc.dram_tensor("x_b", [C, H, W], F32, kind="ExternalInput")
    xhalo = nc.dram_tensor("xhalo", [C, 34, W], F32, kind="ExternalInput")
    wmain_in = nc.dram_tensor("wmain16", [CC, 128, K * O], F16, kind="ExternalInput")
    woff_in = nc.dram_tensor("woff16", [CC, 128, K * 27], F16, kind="ExternalInput")
    b_off_in = nc.dram_tensor("b_off", [27, 1], F32, kind="ExternalInput")
    bprT_in = nc.dram_tensor("bprT", [128, OCC], F32, kind="ExternalInput")
    baseC = nc.dram_tensor("baseC", [128, 16 * 32], F32, kind="ExternalInput")
    ident32 = nc.dram_tensor("ident32", [128, 128], F32, kind="ExternalInput")
    ident16 = nc.dram_tensor("ident16", [128, 128], F16, kind="ExternalInput")

    out_d = nc.dram_tensor("out_c", [O, S], F32, kind="ExternalOutput")

    with tile.TileContext(nc) as tc:
        _build(nc, tc, x_in, xhalo, wmain_in, woff_in, b_off_in, bprT_in,
               baseC, ident32, ident16, out_d)
    nc.compile()
    return nc


def _build(nc, tc, x_in, xhalo, wmain_in, woff_in, b_off_in, bprT_in,
           baseC, ident32, ident16, out_d):
    from contextlib import ExitStack

    with ExitStack() as top:
        pers = top.enter_context(tc.tile_pool(name="pers", bufs=1))
        dram = top.enter_context(tc.tile_pool(name="dram", bufs=1, space="DRAM"))
        quad_t = dram.tile([NTOK, 512], F16, name="quad_scr", tag="quad")
        idxs_t = dram.tile([K, NSB, 16, 64], I16, name="idx_scr", tag="idxs")
        ph1_cm = tc.tile_pool(name="ph1", bufs=1)
        ph1 = ph1_cm.__enter__()

        # ------------- constants -------------
        id32 = pers.tile([128, 128], F32)
        nc.sync.dma_start(out=id32[:], in_=ident32.ap())
        id16 = pers.tile([128, 128], F16)
        nc.sync.dma_start(out=id16[:], in_=ident16.ap())
        base_t = ph1.tile([128, 16, 32], F32)
        nc.sync.dma_start(out=base_t[:], in_=baseC.ap().rearrange("p (a b) -> p a b", a=16))
        boff_t = ph1.tile([27, 1], F32)
        nc.sync.dma_start(out=boff_t[:], in_=b_off_in.ap())
        bprT = pers.tile([128, OCC], F32)
        nc.sync.dma_start(out=bprT[:], in_=bprT_in.ap())
        wmain = []
        for cc in range(CC):
            t = pers.tile([128, K * O], F16, name=f"wmain{cc}", tag=f"wmain{cc}")
            nc.sync.dma_start(out=t[:], in_=wmain_in.ap()[cc])
            wmain.append(t)
        woff16 = []
        for cc in range(CC):
            t = ph1.tile([128, K * 27], F16, name=f"woff{cc}", tag=f"woff{cc}")
            nc.sync.dma_start(out=t[:], in_=woff_in.ap()[cc])
            woff16.append(t)

        # ------------- xom (halo, fp16) + xpad (full, fp16) -------------
        xom = []
        for cc in range(CC):
            t = ph1.tile([128, 34 * W2], F16, name=f"xom{cc}", tag=f"xom{cc}")
            nc.vector.memset(t[:], 0.0)
            dst = t[:].rearrange("p (h w) -> p h w", w=W2)[:, :, P:P + W]
            nc.gpsimd.dma_start(out=dst, in_=xhalo.ap()[cc * 128:(cc + 1) * 128])
            xom.append(t)
        xpad = []
        for cc in range(CC):
            t = ph1.tile([128, NQP], F16, name=f"xpad{cc}", tag=f"xpad{cc}")
            nc.vector.memset(t[:], 0.0)
            dst = t[:, 0:NQ].rearrange("p (h w) -> p h w", w=W2)[:, P:P + H, P:P + W]
            nc.gpsimd.dma_start(out=dst, in_=x_in.ap()[cc * 128:(cc + 1) * 128])
            xpad.append(t)

        # ------------- offset conv: om [27, 2048] -------------
        om_sb = ph1.tile([27, S], F32)
        omT = ph1.tile([128, 16, 32], F32)
        with tc.tile_pool(name="omps", bufs=1, space="PSUM") as omps:
            om_ps = omps.tile([27, S], F32, name="om_ps", tag="om_ps")
            for bk in range(4):           # 4 banks of 512 (8 rows x 64)
                for cc in range(CC):
                    for t9 in range(K):
                        ty, tx = t9 // 3, t9 % 3
                        rhs = xom[cc][:].rearrange("p (h w) -> p h w", w=W2)[
                            :, bk * 8 + ty: bk * 8 + ty + 8,
                            P - 1 + tx: P - 1 + tx + W]
                        nc.tensor.matmul(om_ps[:, bk * 512:(bk + 1) * 512],
                                         woff16[cc][:, t9 * 27:(t9 + 1) * 27], rhs,
                                         start=(cc == 0 and t9 == 0),
                                         stop=(cc == CC - 1 and t9 == K - 1))
            nc.scalar.activation(om_sb[:], om_ps[:], AF.Identity,
                                 bias=boff_t[:, 0:1])

            # ------------- omT [128, 16, 32] -------------
            omT_ps = omps.tile([128, 512], F32, name="omT_ps", tag="omT_ps")
            nc.vector.memset(omT_ps[:], 0.0)
            for ch in range(16):
                nc.tensor.transpose(omT_ps[:, ch * 32: ch * 32 + 27],
                                    om_sb[:, ch * 128:(ch + 1) * 128],
                                    id32[0:27, 0:27])
            nc.vector.tensor_copy(omT[:],
                                  omT_ps[:].rearrange("p (a b) -> p a b", a=16))

        # ------------- sample math -------------
        ppx = ph1.tile([128, 16, 32], F32)
        nc.vector.tensor_tensor(out=ppx[:], in0=omT[:], in1=base_t[:], op=ALU.add)
        ii = ph1.tile([128, 16, 18], I32)
        nc.vector.tensor_copy(ii[:], ppx[:, :, 0:18])
        ff = ph1.tile([128, 16, 18], F32)
        nc.vector.tensor_copy(ff[:], ii[:])
        gtt = ph1.tile([128, 16, 18], F32)
        nc.vector.tensor_tensor(out=gtt[:], in0=ff[:], in1=ppx[:, :, 0:18], op=ALU.is_gt)
        flo = ph1.tile([128, 16, 18], F32)
        nc.vector.tensor_tensor(out=flo[:], in0=ff[:], in1=gtt[:], op=ALU.subtract)
        lf = ph1.tile([128, 16, 18], F32)
        nc.vector.tensor_tensor(out=lf[:], in0=ppx[:, :, 0:18], in1=flo[:], op=ALU.subtract)
        floc = ph1.tile([128, 16, 18], F32)
        nc.vector.tensor_scalar(floc[:], flo[:], 0.0, float(W2 - 2), ALU.max, ALU.min)
        msk = ph1.tile([128, 16, 9], F32)
        nc.scalar.activation(msk[:], omT[:, :, 18:27], AF.Sigmoid)
        ol = ph1.tile([128, 16, 18], F32)
        nc.vector.tensor_scalar(ol[:], lf[:], -1.0, 1.0, ALU.mult, ALU.add)
        # corner weights (with mask folded): [128, 16, 9] each; r order must
        # match the quad token layout: (0,0), (1,0), (0,1), (1,1)
        wr4 = []
        for r, (ya, xa) in enumerate([(0, 0), (1, 0), (0, 1), (1, 1)]):
            yw = ol if ya == 0 else lf     # (1-ly) or ly
            xw = ol if xa == 0 else lf
            wtile = pers.tile([128, 16, 9], F32, name=f"wr4_{r}", tag=f"wr4_{r}")
            nc.vector.tensor_tensor(out=wtile[:], in0=yw[:, :, 0:9],
                                    in1=xw[:, :, 9:18], op=ALU.mult)
            nc.vector.tensor_tensor(out=wtile[:], in0=wtile[:], in1=msk[:], op=ALU.mult)
            wr4.append(wtile)
        # quad token index q = (y0c+1)*W2 + x0c  (front pad of W2 rows)
        qf = ph1.tile([128, 16, 9], F32)
        nc.vector.tensor_scalar(qf[:], floc[:, :, 0:9], float(W2), float(W2),
                                ALU.mult, ALU.add)
        nc.vector.tensor_tensor(out=qf[:], in0=qf[:], in1=floc[:, :, 9:18], op=ALU.add)

        # ------------- gather indices: wrap to [16, 64] + replicate x8 ----
        # wrapped format: token t -> partition t%16, col t//16 with
        # t = chp*128 + s128; sample s128 sits at free pos q*8+jj where
        # s128 = jj*16+q  (so a plain transpose + free-dim permute works).
        idxT16 = ph1.tile([16, K * 128], I16)
        wrapped = []
        with tc.tile_pool(name="idxps", bufs=3, space="PSUM") as idxps:
            for k in range(K):
                tps = idxps.tile([16, 128], F32, name=f"tps{k}", tag="tps")
                nc.tensor.transpose(tps[:], qf[:, :, k:k + 1], id32[:])
                dst = idxT16[:, k * 128:(k + 1) * 128].rearrange(
                    "p (q jj) -> p q jj", q=16)
                src = tps[:].rearrange("p (jj q) -> p q jj", jj=8)
                nc.vector.tensor_copy(dst, src)
            for k in range(K):
                for sblk in range(NSB):
                    src = idxT16[sblk * 8:(sblk + 1) * 8,
                                 k * 128:(k + 1) * 128].rearrange(
                        "p (q jj) -> p q jj", q=16)
                    dstd = idxs_t[k, sblk].rearrange(
                        "q (chp jj) -> q chp jj", chp=8).transpose([1, 0, 2])
                    nc.sync.dma_start(out=dstd, in_=src)
            for sblk in range(NSB):
                wtb = pers.tile([128, K * 64], I16, name=f"wrp{sblk}",
                                tag=f"wrp{sblk}")
                for g in range(8):
                    nc.sync.dma_start(
                        out=wtb[g * 16:(g + 1) * 16, :].rearrange(
                            "q (k c) -> q k c", k=K),
                        in_=idxs_t[:, sblk].transpose([1, 0, 2]))
                wrapped.append(wtb)

        # ------------- XT-quad build -------------
        # quad row r = [XT[r - 74] | XT[r]]; write each transposed XT chunk
        # twice: first halves at rows q+74, second halves at rows q.
        with tc.tile_pool(name="xtp", bufs=2) as xtp, \
             tc.tile_pool(name="xtps", bufs=2, space="PSUM") as xtps:
            for grp in range(11):          # 4 q-chunks per group, 43 chunks
                qcs = list(range(grp * 4, min(grp * 4 + 4, 43)))
                nqc = len(qcs)
                pt = xtps.tile([128, 1024], F16, name=f"xt_ps{grp}", tag="xt_ps")
                for i, qc in enumerate(qcs):
                    for cc in range(CC):
                        nc.tensor.transpose(
                            pt[:, i * 256 + cc * 128: i * 256 + (cc + 1) * 128],
                            xpad[cc][:, qc * 128:(qc + 1) * 128], id16[:])
                st = xtp.tile([128, 1024], F16, name=f"xt_sb{grp}", tag="xt_sb")
                nc.scalar.activation(st[:, 0:nqc * 256], pt[:, 0:nqc * 256], AF.Copy)
                src = st[:, 0:nqc * 256].rearrange("p (qc c) -> p qc c", c=256)
                d1 = quad_t[W2 + grp * 512: W2 + grp * 512 + nqc * 128,
                            0:256].rearrange("(qc p) c -> p qc c", p=128)
                nc.sync.dma_start(out=d1, in_=src)
                d2 = quad_t[grp * 512: grp * 512 + nqc * 128,
                            256:512].rearrange("(qc p) c -> p qc c", p=128)
                nc.sync.dma_start(out=d2, in_=src)

        # gather source AP: overlapping rows (stride 512 elems, len 1024)
        quad_g = quad_t[:, :].copy()
        quad_g.ap[0] = [512, NTOK - 1]
        quad_g.ap[1] = [1, 1024]

        # ------------- main loop -------------
        ph1_cm.__exit__(None, None, None)

        with tc.tile_pool(name="mg", bufs=2) as mg, \
             tc.tile_pool(name="mv", bufs=2) as mv, \
             tc.tile_pool(name="mvs", bufs=2) as mvs, \
             tc.tile_pool(name="mps", bufs=2, space="PSUM") as mps, \
             tc.tile_pool(name="accp", bufs=1, space="PSUM") as accp, \
             tc.tile_pool(name="outp", bufs=2) as outp:
            for sblk in range(NSB):
                vsb = [[None] * CC for _ in range(K)]
                for k in range(K):
                    gt = mg.tile([128, CHB, 1024], F16, name=f"g{sblk}_{k}",
                                 tag="gt")
                    nc.gpsimd.dma_gather(gt[:], quad_g,
                                         wrapped[sblk][:, k * 64:(k + 1) * 64],
                                         NI, NI, 1024, elem_step=512,
                                         single_packet=False)
                    vt = mv.tile([128, CHB * 256], F16, name=f"v{sblk}_{k}",
                                 tag="vt")
                    for chp in range(CHB):
                        ch = sblk * CHB + chp
                        vts = vt[:, chp * 256:(chp + 1) * 256]
                        nc.vector.tensor_scalar(
                            vts, gt[:, chp, 0:256],
                            wr4[0][:, ch, k:k + 1], None, ALU.mult)
                        for r in range(1, 4):
                            nc.vector.scalar_tensor_tensor(
                                out=vts, in0=gt[:, chp, r * 256:(r + 1) * 256],
                                scalar=wr4[r][:, ch, k:k + 1], in1=vts,
                                op0=ALU.mult, op1=ALU.add)
                    # transpose V^T -> V [c, s]
                    vps = [mps.tile([128, 1024], F16, name=f"vps{sblk}_{k}_{cc}",
                                    tag=f"vps{cc}") for cc in range(CC)]
                    for chp in range(CHB):
                        for cc in range(CC):
                            nc.tensor.transpose(
                                vps[cc][:, chp * 128:(chp + 1) * 128],
                                vt[:, chp * 256 + cc * 128: chp * 256 + (cc + 1) * 128],
                                id16[:])
                    for cc in range(CC):
                        t = mvs.tile([128, 1024], F16, name=f"vsb{sblk}_{k}_{cc}",
                                     tag=f"vsb{k}_{cc}")
                        nc.scalar.activation(t[:], vps[cc][:], AF.Copy)
                        vsb[k][cc] = t
                # weight-stationary matmuls: out^T[o, s] over (k, cc) in PSUM
                for occ in range(OCC):
                    for sc in range(SB // 512):
                        acc = accp.tile([128, 512], F32,
                                        name=f"acc{sblk}_{occ}_{sc}",
                                        tag=f"acc{occ}_{sc}")
                        for k in range(K):
                            for cc in range(CC):
                                nc.tensor.matmul(
                                    acc[:],
                                    wmain[cc][:, k * O + occ * 128: k * O + occ * 128 + 128],
                                    vsb[k][cc][:, sc * 512:(sc + 1) * 512],
                                    start=(k == 0 and cc == 0),
                                    stop=(k == K - 1 and cc == CC - 1))
                        osb = outp.tile([128, 512], F32,
                                        name=f"osb{sblk}_{occ}_{sc}", tag="osb")
                        nc.scalar.activation(osb[:], acc[:], AF.Relu,
                                             bias=bprT[:, occ:occ + 1])
                        nc.sync.dma_start(
                            out=out_d.ap()[occ * 128:(occ + 1) * 128,
                                           sblk * SB + sc * 512:
                                           sblk * SB + (sc + 1) * 512],
                            in_=osb[:])


# ===================== host side =====================

def _host_prep(inputs):
    """Per-core input maps: layout + BN/bias folding (no x-dependent work)."""
    x = np.ascontiguousarray(inputs["x"], dtype=np.float32)
    w_off = np.asarray(inputs["w_off"], np.float32)
    b_off = np.asarray(inputs["b_off"], np.float32)
    weight = np.asarray(inputs["weight"], np.float32)
    bias = np.asarray(inputs["bias"], np.float32)
    gamma = np.asarray(inputs["gamma"], np.float32)
    beta = np.asarray(inputs["beta"], np.float32)
    run_mean = np.asarray(inputs["run_mean"], np.float32)
    run_var = np.asarray(inputs["run_var"], np.float32)

    # BN fold: W' = W * sfac[o];  b' = sfac*(bias - mean) + beta
    sfac = gamma / np.sqrt(run_var + EPS)
    bpr = sfac * (bias - run_mean) + beta
    wsc = weight.reshape(O, C, K) * sfac[:, None, None]
    # [O, C, K] -> [C, K, O] -> [CC, 128, K*O], fp16
    wt = wsc.transpose(1, 2, 0).reshape(CC, 128, K * O).astype(np.float16)
    wt = np.ascontiguousarray(wt)
    wofft = w_off.reshape(27, C, K).transpose(1, 2, 0).reshape(
        CC, 128, K * 27).astype(np.float16)
    wofft = np.ascontiguousarray(wofft)
    bprT = np.ascontiguousarray(bpr.reshape(OCC, 128).T.astype(np.float32))
    id32 = np.eye(128, dtype=np.float32)
    id16 = np.eye(128, dtype=np.float16)
    boff = b_off.reshape(27, 1).astype(np.float32)

    in_maps = []
    for core in range(N_CORES):
        b, half = core // 2, core % 2
        h0 = half * HH
        # halo rows [h0-1, h0+33) with zero pad at the image boundary
        halo = np.zeros((C, 34, W), np.float32)
        lo, hi = h0 - 1, h0 + 33
        slo, shi = max(lo, 0), min(hi, H)
        halo[:, slo - lo: slo - lo + (shi - slo)] = x[b, :, slo:shi]
        # baseC [128, 16, 32]: cols 0-8 pyP base, 9-17 pxP base, rest 0
        basec = np.zeros((128, 16, 32), np.float32)
        pp_ = np.arange(128)
        for ch in range(16):
            s_ = ch * 128 + pp_
            hloc = h0 + s_ // W
            wloc = s_ % W
            for k in range(K):
                basec[:, ch, k] = hloc + (k // 3) - 1 + P
                basec[:, ch, 9 + k] = wloc + (k % 3) - 1 + P
        in_maps.append({
            "x_b": np.ascontiguousarray(x[b]),
            "xhalo": halo,
            "wmain16": wt,
            "woff16": wofft,
            "b_off": boff,
            "bprT": bprT,
            "baseC": basec.reshape(128, 16 * 32),
            "ident32": id32,
            "ident16": id16,
        })
    return in_maps


def _get_nc():
    if "nc" not in _NC_CACHE:
        _NC_CACHE["nc"] = build_nc()
    return _NC_CACHE["nc"]


def kernel(**inputs):
    nc = _get_nc()
    in_maps = _host_prep(inputs)
    res = bass_utils.run_bass_kernel_spmd(nc, in_maps, core_ids=list(range(N_CORES)))
    out = np.zeros((B, O, H, W), np.float32)
    for core in range(N_CORES):
        b, half = core // 2, core % 2
        out[b, :, half * HH:(half + 1) * HH, :] = (
            res.results[core]["out_c"].reshape(O, HH, W))
    return out


# revision 6
# speedup vs baseline: 2.3489x; 1.0320x over previous
"""Trainium2 Bass kernel: modulated deformable conv 3x3 (DCNv2) + BN(eval)
+ ReLU.  B=4, C=O=256, H=W=64, distributed over 8 NeuronCores.

Sharding: core i -> batch b = i//2, image row-half = i%2 (32 rows). Each core
computes out[b, :, h0:h0+32, :] fully.

v2 design (quad-token gather):
  - xpad [C,74*74] fp16 zero-padded image (pad P=5) via casting SWDGE DMA;
    xom [C,34*74] halo rows for the offset conv (host-sliced per core).
  - offset conv om[27,2048] (9 shifted matmuls x 2 cc, 4 PSUM banks).
  - omT -> bilinear corner weights wr4 (kept as per-partition scalars) and
    ONE int16 token index per (sample, tap): q = (y0+1)*74 + x0 (clamped to
    the zero pad, so out-of-image corners contribute 0).
  - XT-quad DRAM scratch [NTOK,512] fp16: row r = [X^T[r-74] | X^T[r]].  A
    single 2048B descriptor starting at row r covers rows r,r+1 = all four
    bilinear corners of a sample: 1 descriptor per (sample, tap) instead of
    4, quartering SWDGE descriptor-generation time (the v1 bottleneck).
  - per (sblk, k): one dma_gather (NI=1024, elem 2048B, elem_step 1024B) ->
    gt [128s, 8chp, 1024]; DVE combines the 4 corners (tensor_scalar + 3
    fused scalar_tensor_tensor with per-partition weights); PE transposes
    V^T -> V[c,s]; weight-stationary matmuls accumulate out^T[o,s] in PSUM
    over (k, cc); Scalar engine applies folded BN bias + ReLU from PSUM;
    DMA out^T[o,s] fp32.
BN + conv bias are folded on host: W' = W * (gamma*rsqrt(var+eps)) ;
b' = s*(bias-mean)+beta, applied as per-partition activation bias.
"""

import numpy as np

import concourse.bass as bass
import concourse.bacc as bacc
import concourse.mybir as mybir
import concourse.tile as tile
from concourse import bass_utils, library_config

F32 = mybir.dt.float32
F16 = mybir.dt.float16
I16 = mybir.dt.int16
I32 = mybir.dt.int32
AF = mybir.ActivationFunctionType
ALU = mybir.AluOpType

B, C, O, H, W = 4, 256, 256, 64, 64
K = 9
P = 5
W2 = H + 2 * P            # 74
NQ = W2 * W2              # 5476
NQP = 5504                # 43*128 (transposed q count, padded)
NTOK = 74 + NQP           # 5578 quad rows (front pad of 74)
HH = 32                   # rows per core
S = HH * W                # 2048 samples per core
CC = C // 128             # 2
OCC = O // 128            # 2
NSB = 2                   # sample blocks
SB = S // NSB             # 1024 samples / block
CHB = 8                   # 128-sample chunks per block
NI = SB                   # gather indices per call (1 per sample)
EPS = 1e-5
N_CORES = 8

_NC_CACHE = {}


def build_nc():
    nc = bacc.Bacc("TRN2", target_bir_lowering=False, debug=False,
                   num_devices=N_CORES)

    x_in = nc.dram_tensor("x_b", [C, H, W], F32, kind="ExternalInput")
    xhalo = nc.dram_tensor("xhalo", [C, 34, W], F32, kind="ExternalInput")
    wmain_in = nc.dram_tensor("wmain16", [CC, 128, K * O], F16, kind="ExternalInput")
    woff_in = nc.dram_tensor("woff16", [CC, 128, K * 27], F16, kind="ExternalInput")
    b_off_in = nc.dram_tensor("b_off", [27, 1], F32, kind="ExternalInput")
    bprT_in = nc.dram_tensor("bprT", [128, OCC], F32, kind="ExternalInput")
    baseC = nc.dram_tensor("baseC", [128, 16 * 32], F32, kind="ExternalInput")
    ident32 = nc.dram_tensor("ident32", [128, 128], F32, kind="ExternalInput")
    ident16 = nc.dram_tensor("ident16", [128, 128], F16, kind="ExternalInput")

    out_d = nc.dram_tensor("out_c", [O, S], F32, kind="ExternalOutput")

    with tile.TileContext(nc) as tc:
        _build(nc, tc, x_in, xhalo, wmain_in, woff_in, b_off_in, bprT_in,
               baseC, ident32, ident16, out_d)
    nc.compile()
    return nc


def _build(nc, tc, x_in, xhalo, wmain_in, woff_in, b_off_in, bprT_in,
           baseC, ident32, ident16, out_d):
    from contextlib import ExitStack

    with ExitStack() as top:
        pers = top.enter_context(tc.tile_pool(name="pers", bufs=1))
        dram = top.enter_context(tc.tile_pool(name="dram", bufs=1, space="DRAM"))
        quad_t = dram.tile([NTOK, 512], F16, name="quad_scr", tag="quad")
        idxs_t = dram.tile([K, NSB, 16, 64], I16, name="idx_scr", tag="idxs")
        ph1_cm = tc.tile_pool(name="ph1", bufs=1)
        ph1 = ph1_cm.__enter__()

        # ------------- constants -------------
        id32 = pers.tile([128, 128], F32)
        nc.sync.dma_start(out=id32[:], in_=ident32.ap())
        id16 = pers.tile([128, 128], F16)
        nc.sync.dma_start(out=id16[:], in_=ident16.ap())
        base_t = ph1.tile([128, 16, 32], F32)
        nc.sync.dma_start(out=base_t[:], in_=baseC.ap().rearrange("p (a b) -> p a b", a=16))
        boff_t = ph1.tile([27, 1], F32)
        nc.sync.dma_start(out=boff_t[:], in_=b_off_in.ap())
        bprT = pers.tile([128, OCC], F32)
        nc.sync.dma_start(out=bprT[:], in_=bprT_in.ap())
        wmain = []
        for cc in range(CC):
            t = pers.tile([128, K * O], F16, name=f"wmain{cc}", tag=f"wmain{cc}")
            nc.sync.dma_start(out=t[:], in_=wmain_in.ap()[cc])
            wmain.append(t)
        woff16 = []
        for cc in range(CC):
            t = ph1.tile([128, K * 27], F16, name=f"woff{cc}", tag=f"woff{cc}")
            nc.sync.dma_start(out=t[:], in_=woff_in.ap()[cc])
            woff16.append(t)

        # ------------- xom (halo, fp16) + xpad (full, fp16) -------------
        xom = []
        for cc in range(CC):
            t = ph1.tile([128, 34 * W2], F16, name=f"xom{cc}", tag=f"xom{cc}")
            nc.vector.memset(t[:], 0.0)
            dst = t[:].rearrange("p (h w) -> p h w", w=W2)[:, :, P:P + W]
            nc.gpsimd.dma_start(out=dst, in_=xhalo.ap()[cc * 128:(cc + 1) * 128])
            xom.append(t)
        xpad = []
        for cc in range(CC):
            t = ph1.tile([128, NQP], F16, name=f"xpad{cc}", tag=f"xpad{cc}")
            nc.vector.memset(t[:], 0.0)
            dst = t[:, 0:NQ].rearrange("p (h w) -> p h w", w=W2)[:, P:P + H, P:P + W]
            nc.gpsimd.dma_start(out=dst, in_=x_in.ap()[cc * 128:(cc + 1) * 128])
            xpad.append(t)

        # ------------- offset conv: om [27, 2048] -------------
        om_sb = ph1.tile([27, S], F32)
        omT = ph1.tile([128, 16, 32], F32)
        with tc.tile_pool(name="omps", bufs=1, space="PSUM") as omps:
            om_ps = omps.tile([27, S], F32, name="om_ps", tag="om_ps")
            for bk in range(4):           # 4 banks of 512 (8 rows x 64)
                for cc in range(CC):
                    for t9 in range(K):
                        ty, tx = t9 // 3, t9 % 3
                        rhs = xom[cc][:].rearrange("p (h w) -> p h w", w=W2)[
                            :, bk * 8 + ty: bk * 8 + ty + 8,
                            P - 1 + tx: P - 1 + tx + W]
                        nc.tensor.matmul(om_ps[:, bk * 512:(bk + 1) * 512],
                                         woff16[cc][:, t9 * 27:(t9 + 1) * 27], rhs,
                                         start=(cc == 0 and t9 == 0),
                                         stop=(cc == CC - 1 and t9 == K - 1))
            nc.scalar.activation(om_sb[:], om_ps[:], AF.Identity,
                                 bias=boff_t[:, 0:1])

            # ------------- omT [128, 16, 32] -------------
            omT_ps = omps.tile([128, 512], F32, name="omT_ps", tag="omT_ps")
            nc.vector.memset(omT_ps[:], 0.0)
            for ch in range(16):
                nc.tensor.transpose(omT_ps[:, ch * 32: ch * 32 + 27],
                                    om_sb[:, ch * 128:(ch + 1) * 128],
                                    id32[0:27, 0:27])
            nc.vector.tensor_copy(omT[:],
                                  omT_ps[:].rearrange("p (a b) -> p a b", a=16))

        # ------------- sample math -------------
        ppx = ph1.tile([128, 16, 32], F32)
        nc.vector.tensor_tensor(out=ppx[:], in0=omT[:], in1=base_t[:], op=ALU.add)
        ii = ph1.tile([128, 16, 18], I32)
        nc.vector.tensor_copy(ii[:], ppx[:, :, 0:18])
        ff = ph1.tile([128, 16, 18], F32)
        nc.vector.tensor_copy(ff[:], ii[:])
        gtt = ph1.tile([128, 16, 18], F32)
        nc.vector.tensor_tensor(out=gtt[:], in0=ff[:], in1=ppx[:, :, 0:18], op=ALU.is_gt)
        flo = ph1.tile([128, 16, 18], F32)
        nc.vector.tensor_tensor(out=flo[:], in0=ff[:], in1=gtt[:], op=ALU.subtract)
        lf = ph1.tile([128, 16, 18], F32)
        nc.vector.tensor_tensor(out=lf[:], in0=ppx[:, :, 0:18], in1=flo[:], op=ALU.subtract)
        floc = ph1.tile([128, 16, 18], F32)
        nc.vector.tensor_scalar(floc[:], flo[:], 0.0, float(W2 - 2), ALU.max, ALU.min)
        msk = ph1.tile([128, 16, 9], F32)
        nc.scalar.activation(msk[:], omT[:, :, 18:27], AF.Sigmoid)
        ol = ph1.tile([128, 16, 18], F32)
        nc.vector.tensor_scalar(ol[:], lf[:], -1.0, 1.0, ALU.mult, ALU.add)
        # corner weights (with mask folded): [128, 16, 9] each; r order must
        # match the quad token layout: (0,0), (1,0), (0,1), (1,1)
        wr4 = []
        for r, (ya, xa) in enumerate([(0, 0), (1, 0), (0, 1), (1, 1)]):
            yw = ol if ya == 0 else lf     # (1-ly) or ly
            xw = ol if xa == 0 else lf
            wtile = pers.tile([128, 16, 9], F32, name=f"wr4_{r}", tag=f"wr4_{r}")
            nc.vector.tensor_tensor(out=wtile[:], in0=yw[:, :, 0:9],
                                    in1=xw[:, :, 9:18], op=ALU.mult)
            nc.vector.tensor_tensor(out=wtile[:], in0=wtile[:], in1=msk[:], op=ALU.mult)
            wr4.append(wtile)
        # quad token index q = (y0c+1)*W2 + x0c  (front pad of W2 rows)
        qf = ph1.tile([128, 16, 9], F32)
        nc.vector.tensor_scalar(qf[:], floc[:, :, 0:9], float(W2), float(W2),
                                ALU.mult, ALU.add)
        nc.vector.tensor_tensor(out=qf[:], in0=qf[:], in1=floc[:, :, 9:18], op=ALU.add)

        # ------------- gather indices: wrap to [16, 64] + replicate x8 ----
        # wrapped format: token t -> partition t%16, col t//16 with
        # t = chp*128 + s128; sample s128 sits at free pos q*8+jj where
        # s128 = jj*16+q  (so a plain transpose + free-dim permute works).
        idxT16 = ph1.tile([16, K * 128], I16)
        wrapped = []
        with tc.tile_pool(name="idxps", bufs=3, space="PSUM") as idxps:
            for k in range(K):
                tps = idxps.tile([16, 128], F32, name=f"tps{k}", tag="tps")
                nc.tensor.transpose(tps[:], qf[:, :, k:k + 1], id32[:])
                dst = idxT16[:, k * 128:(k + 1) * 128].rearrange(
                    "p (q jj) -> p q jj", q=16)
                src = tps[:].rearrange("p (jj q) -> p q jj", jj=8)
                nc.vector.tensor_copy(dst, src)
            for k in range(K):
                for sblk in range(NSB):
                    src = idxT16[sblk * 8:(sblk + 1) * 8,
                                 k * 128:(k + 1) * 128].rearrange(
                        "p (q jj) -> p q jj", q=16)
                    dstd = idxs_t[k, sblk].rearrange(
                        "q (chp jj) -> q chp jj", chp=8).transpose([1, 0, 2])
                    nc.sync.dma_start(out=dstd, in_=src)
            for sblk in range(NSB):
                wtb = pers.tile([128, K * 64], I16, name=f"wrp{sblk}",
                                tag=f"wrp{sblk}")
                for g in range(8):
                    nc.sync.dma_start(
                        out=wtb[g * 16:(g + 1) * 16, :].rearrange(
                            "q (k c) -> q k c", k=K),
                        in_=idxs_t[:, sblk].transpose([1, 0, 2]))
                wrapped.append(wtb)

        # ------------- XT-quad build -------------
        # quad row r = [XT[r - 74] | XT[r]]; write each transposed XT chunk
        # twice: first halves at rows q+74, second halves at rows q.
        with tc.tile_pool(name="xtp", bufs=2) as xtp, \
             tc.tile_pool(name="xtps", bufs=2, space="PSUM") as xtps:
            for grp in range(11):          # 4 q-chunks per group, 43 chunks
                qcs = list(range(grp * 4, min(grp * 4 + 4, 43)))
                nqc = len(qcs)
                pt = xtps.tile([128, 1024], F16, name=f"xt_ps{grp}", tag="xt_ps")
                for i, qc in enumerate(qcs):
                    for cc in range(CC):
                        nc.tensor.transpose(
                            pt[:, i * 256 + cc * 128: i * 256 + (cc + 1) * 128],
                            xpad[cc][:, qc * 128:(qc + 1) * 128], id16[:])
                st = xtp.tile([128, 1024], F16, name=f"xt_sb{grp}", tag="xt_sb")
                nc.scalar.activation(st[:, 0:nqc * 256], pt[:, 0:nqc * 256], AF.Copy)
                src = st[:, 0:nqc * 256].rearrange("p (qc c) -> p qc c", c=256)
                d1 = quad_t[W2 + grp * 512: W2 + grp * 512 + nqc * 128,
                            0:256].rearrange("(qc p) c -> p qc c", p=128)
                nc.sync.dma_start(out=d1, in_=src)
                d2 = quad_t[grp * 512: grp * 512 + nqc * 128,
                            256:512].rearrange("(qc p) c -> p qc c", p=128)
                nc.sync.dma_start(out=d2, in_=src)

        # gather source AP: overlapping rows (stride 512 elems, len 1024)
        quad_g = quad_t[:, :].copy()
        quad_g.ap[0] = [512, NTOK - 1]
        quad_g.ap[1] = [1, 1024]

        # ------------- main loop -------------
        ph1_cm.__exit__(None, None, None)

        POOL_CHP = ()         # per-partition-scalar ops are DVE-only (Pool ISA
                              # lacks TensorScalarPtr) — keep combine on DVE
        with tc.tile_pool(name="mg", bufs=3) as mg, \
             tc.tile_pool(name="mv", bufs=2) as mv, \
             tc.tile_pool(name="mvs", bufs=2) as mvs, \
             tc.tile_pool(name="mps", bufs=2, space="PSUM") as mps, \
             tc.tile_pool(name="accp", bufs=1, space="PSUM") as accp, \
             tc.tile_pool(name="outp", bufs=2) as outp:
            for sblk in range(NSB):
                accs = {}
                for occ in range(OCC):
                    for sc in range(SB // 512):
                        accs[(occ, sc)] = accp.tile(
                            [128, 512], F32, name=f"acc{sblk}_{occ}_{sc}",
                            tag=f"acc{occ}_{sc}")
                for k in range(K):
                    gt = mg.tile([128, CHB, 1024], F16, name=f"g{sblk}_{k}",
                                 tag="gt")
                    nc.gpsimd.dma_gather(gt[:], quad_g,
                                         wrapped[sblk][:, k * 64:(k + 1) * 64],
                                         NI, NI, 1024, elem_step=512,
                                         single_packet=False)
                    vt = mv.tile([128, CHB * 256], F16, name=f"v{sblk}_{k}",
                                 tag="vt")
                    for chp in range(CHB):
                        ch = sblk * CHB + chp
                        eng = nc.gpsimd if chp in POOL_CHP else nc.vector
                        vts = vt[:, chp * 256:(chp + 1) * 256]
                        eng.tensor_scalar(
                            vts, gt[:, chp, 0:256],
                            wr4[0][:, ch, k:k + 1], None, ALU.mult)
                        for r in range(1, 4):
                            eng.scalar_tensor_tensor(
                                out=vts, in0=gt[:, chp, r * 256:(r + 1) * 256],
                                scalar=wr4[r][:, ch, k:k + 1], in1=vts,
                                op0=ALU.mult, op1=ALU.add)
                    # transpose V^T -> V [c, s]
                    vps = [mps.tile([128, 1024], F16, name=f"vps{sblk}_{k}_{cc}",
                                    tag=f"vps{cc}") for cc in range(CC)]
                    for chp in range(CHB):
                        for cc in range(CC):
                            nc.tensor.transpose(
                                vps[cc][:, chp * 128:(chp + 1) * 128],
                                vt[:, chp * 256 + cc * 128: chp * 256 + (cc + 1) * 128],
                                id16[:])
                    vsbk = []
                    for cc in range(CC):
                        t = mvs.tile([128, 1024], F16, name=f"vsb{sblk}_{k}_{cc}",
                                     tag=f"vsb{k}_{cc}")
                        nc.scalar.activation(t[:], vps[cc][:], AF.Copy)
                        vsbk.append(t)
                    # incremental weight-stationary matmuls for this tap
                    for occ in range(OCC):
                        for sc in range(SB // 512):
                            for cc in range(CC):
                                nc.tensor.matmul(
                                    accs[(occ, sc)][:],
                                    wmain[cc][:, k * O + occ * 128: k * O + occ * 128 + 128],
                                    vsbk[cc][:, sc * 512:(sc + 1) * 512],
                                    start=(k == 0 and cc == 0),
                                    stop=(k == K - 1 and cc == CC - 1),
                                    skip_group_check=True)
                for occ in range(OCC):
                    for sc in range(SB // 512):
                        osb = outp.tile([128, 512], F32,
                                        name=f"osb{sblk}_{occ}_{sc}", tag="osb")
                        nc.scalar.activation(osb[:], accs[(occ, sc)][:], AF.Relu,
                                             bias=bprT[:, occ:occ + 1])
                        nc.sync.dma_start(
                            out=out_d.ap()[occ * 128:(occ + 1) * 128,
                                           sblk * SB + sc * 512:
                                           sblk * SB + (sc + 1) * 512],
                            in_=osb[:])


# ===================== host side =====================

def _host_prep(inputs):
    """Per-core input maps: layout + BN/bias folding (no x-dependent work)."""
    x = np.ascontiguousarray(inputs["x"], dtype=np.float32)
    w_off = np.asarray(inputs["w_off"], np.float32)
    b_off = np.asarray(inputs["b_off"], np.float32)
    weight = np.asarray(inputs["weight"], np.float32)
    bias = np.asarray(inputs["bias"], np.float32)
    gamma = np.asarray(inputs["gamma"], np.float32)
    beta = np.asarray(inputs["beta"], np.float32)
    run_mean = np.asarray(inputs["run_mean"], np.float32)
    run_var = np.asarray(inputs["run_var"], np.float32)

    # BN fold: W' = W * sfac[o];  b' = sfac*(bias - mean) + beta
    sfac = gamma / np.sqrt(run_var + EPS)
    bpr = sfac * (bias - run_mean) + beta
    wsc = weight.reshape(O, C, K) * sfac[:, None, None]
    # [O, C, K] -> [C, K, O] -> [CC, 128, K*O], fp16
    wt = wsc.transpose(1, 2, 0).reshape(CC, 128, K * O).astype(np.float16)
    wt = np.ascontiguousarray(wt)
    wofft = w_off.reshape(27, C, K).transpose(1, 2, 0).reshape(
        CC, 128, K * 27).astype(np.float16)
    wofft = np.ascontiguousarray(wofft)
    bprT = np.ascontiguousarray(bpr.reshape(OCC, 128).T.astype(np.float32))
    id32 = np.eye(128, dtype=np.float32)
    id16 = np.eye(128, dtype=np.float16)
    boff = b_off.reshape(27, 1).astype(np.float32)

    in_maps = []
    for core in range(N_CORES):
        b, half = core // 2, core % 2
        h0 = half * HH
        # halo rows [h0-1, h0+33) with zero pad at the image boundary
        halo = np.zeros((C, 34, W), np.float32)
        lo, hi = h0 - 1, h0 + 33
        slo, shi = max(lo, 0), min(hi, H)
        halo[:, slo - lo: slo - lo + (shi - slo)] = x[b, :, slo:shi]
        # baseC [128, 16, 32]: cols 0-8 pyP base, 9-17 pxP base, rest 0
        basec = np.zeros((128, 16, 32), np.float32)
        pp_ = np.arange(128)
        for ch in range(16):
            s_ = ch * 128 + pp_
            hloc = h0 + s_ // W
            wloc = s_ % W
            for k in range(K):
                basec[:, ch, k] = hloc + (k // 3) - 1 + P
                basec[:, ch, 9 + k] = wloc + (k % 3) - 1 + P
        in_maps.append({
            "x_b": np.ascontiguousarray(x[b]),
            "xhalo": halo,
            "wmain16": wt,
            "woff16": wofft,
            "b_off": boff,
            "bprT": bprT,
            "baseC": basec.reshape(128, 16 * 32),
            "ident32": id32,
            "ident16": id16,
        })
    return in_maps


def _get_nc():
    if "nc" not in _NC_CACHE:
        _NC_CACHE["nc"] = build_nc()
    return _NC_CACHE["nc"]


def kernel(**inputs):
    nc = _get_nc()
    in_maps = _host_prep(inputs)
    res = bass_utils.run_bass_kernel_spmd(nc, in_maps, core_ids=list(range(N_CORES)))
    out = np.zeros((B, O, H, W), np.float32)
    for core in range(N_CORES):
        b, half = core // 2, core % 2
        out[b, :, half * HH:(half + 1) * HH, :] = (
            res.results[core]["out_c"].reshape(O, HH, W))
    return out


# revision 8
# speedup vs baseline: 2.4882x; 1.0593x over previous
"""Trainium2 Bass kernel: modulated deformable conv 3x3 (DCNv2) + BN(eval)
+ ReLU.  B=4, C=O=256, H=W=64, distributed over 8 NeuronCores.

Sharding: core i -> batch b = i//2, image row-half = i%2 (32 rows). Each core
computes out[b, :, h0:h0+32, :] fully.

v2 design (quad-token gather):
  - xpad [C,74*74] fp16 zero-padded image (pad P=5) via casting SWDGE DMA;
    xom [C,34*74] halo rows for the offset conv (host-sliced per core).
  - offset conv om[27,2048] (9 shifted matmuls x 2 cc, 4 PSUM banks).
  - omT -> bilinear corner weights wr4 (kept as per-partition scalars) and
    ONE int16 token index per (sample, tap): q = (y0+1)*74 + x0 (clamped to
    the zero pad, so out-of-image corners contribute 0).
  - XT-quad DRAM scratch [NTOK,512] fp16: row r = [X^T[r-74] | X^T[r]].  A
    single 2048B descriptor starting at row r covers rows r,r+1 = all four
    bilinear corners of a sample: 1 descriptor per (sample, tap) instead of
    4, quartering SWDGE descriptor-generation time (the v1 bottleneck).
  - per (sblk, k): one dma_gather (NI=1024, elem 2048B, elem_step 1024B) ->
    gt [128s, 8chp, 1024]; DVE combines the 4 corners (tensor_scalar + 3
    fused scalar_tensor_tensor with per-partition weights); PE transposes
    V^T -> V[c,s]; weight-stationary matmuls accumulate out^T[o,s] in PSUM
    over (k, cc); Scalar engine applies folded BN bias + ReLU from PSUM;
    DMA out^T[o,s] fp32.
BN + conv bias are folded on host: W' = W * (gamma*rsqrt(var+eps)) ;
b' = s*(bias-mean)+beta, applied as per-partition activation bias.
"""

import numpy as np

import concourse.bass as bass
import concourse.bacc as bacc
import concourse.mybir as mybir
import concourse.tile as tile
from concourse import bass_utils, library_config

F32 = mybir.dt.float32
F16 = mybir.dt.float16
I16 = mybir.dt.int16
I32 = mybir.dt.int32
AF = mybir.ActivationFunctionType
ALU = mybir.AluOpType

B, C, O, H, W = 4, 256, 256, 64, 64
K = 9
P = 5
W2 = H + 2 * P            # 74
NQ = W2 * W2              # 5476
NQP = 5504                # 43*128 (transposed q count, padded)
NTOK = 74 + NQP           # 5578 quad rows (front pad of 74)
HH = 32                   # rows per core
S = HH * W                # 2048 samples per core
CC = C // 128             # 2
OCC = O // 128            # 2
NSB = 2                   # sample blocks
SB = S // NSB             # 1024 samples / block
CHB = 8                   # 128-sample chunks per block
NI = SB                   # gather indices per call (1 per sample)
EPS = 1e-5
N_CORES = 8

_NC_CACHE = {}


def build_nc():
    nc = bacc.Bacc("TRN2", target_bir_lowering=False, debug=False,
                   num_devices=N_CORES)

    x_in = nc.dram_tensor("x_b", [C, H, W], F32, kind="ExternalInput")
    xhalo = nc.dram_tensor("xhalo", [C, 34, W], F32, kind="ExternalInput")
    wmain_in = nc.dram_tensor("wmain16", [CC, 128, K * O], F16, kind="ExternalInput")
    woff_in = nc.dram_tensor("woff16", [CC, 128, K * 27], F16, kind="ExternalInput")
    b_off_in = nc.dram_tensor("b_off", [27, 1], F32, kind="ExternalInput")
    bprT_in = nc.dram_tensor("bprT", [128, OCC], F32, kind="ExternalInput")
    baseC = nc.dram_tensor("baseC", [128, 16 * 32], F32, kind="ExternalInput")
    ident32 = nc.dram_tensor("ident32", [128, 128], F32, kind="ExternalInput")
    ident16 = nc.dram_tensor("ident16", [128, 128], F16, kind="ExternalInput")

    out_d = nc.dram_tensor("out_c", [O, S], F32, kind="ExternalOutput")

    with tile.TileContext(nc) as tc:
        _build(nc, tc, x_in, xhalo, wmain_in, woff_in, b_off_in, bprT_in,
               baseC, ident32, ident16, out_d)
    nc.compile()
    return nc


def _build(nc, tc, x_in, xhalo, wmain_in, woff_in, b_off_in, bprT_in,
           baseC, ident32, ident16, out_d):
    from contextlib import ExitStack

    with ExitStack() as top:
        pers = top.enter_context(tc.tile_pool(name="pers", bufs=1))
        dram = top.enter_context(tc.tile_pool(name="dram", bufs=1, space="DRAM"))
        quad_t = dram.tile([NTOK, 512], F16, name="quad_scr", tag="quad")
        idxs_t = dram.tile([K, NSB, 16, 64], I16, name="idx_scr", tag="idxs")
        ph1_cm = tc.tile_pool(name="ph1", bufs=1)
        ph1 = ph1_cm.__enter__()

        # ------------- constants -------------
        id32 = pers.tile([128, 128], F32)
        nc.sync.dma_start(out=id32[:], in_=ident32.ap())
        id16 = pers.tile([128, 128], F16)
        nc.sync.dma_start(out=id16[:], in_=ident16.ap())
        base_t = ph1.tile([128, 16, 32], F32)
        nc.sync.dma_start(out=base_t[:], in_=baseC.ap().rearrange("p (a b) -> p a b", a=16))
        boff_t = ph1.tile([27, 1], F32)
        nc.sync.dma_start(out=boff_t[:], in_=b_off_in.ap())
        bprT = pers.tile([128, OCC], F32)
        nc.sync.dma_start(out=bprT[:], in_=bprT_in.ap())
        wmain = []
        for cc in range(CC):
            t = pers.tile([128, K * O], F16, name=f"wmain{cc}", tag=f"wmain{cc}")
            nc.sync.dma_start(out=t[:], in_=wmain_in.ap()[cc])
            wmain.append(t)
        woff16 = []
        for cc in range(CC):
            t = ph1.tile([128, K * 27], F16, name=f"woff{cc}", tag=f"woff{cc}")
            nc.sync.dma_start(out=t[:], in_=woff_in.ap()[cc])
            woff16.append(t)

        # ------------- xom (halo, fp16) + xpad (full, fp16) -------------
        xom = []
        for cc in range(CC):
            t = ph1.tile([128, 34 * W2], F16, name=f"xom{cc}", tag=f"xom{cc}")
            nc.vector.memset(t[:], 0.0)
            dst = t[:].rearrange("p (h w) -> p h w", w=W2)[:, :, P:P + W]
            nc.gpsimd.dma_start(out=dst, in_=xhalo.ap()[cc * 128:(cc + 1) * 128])
            xom.append(t)
        xpad = []
        for cc in range(CC):
            t = ph1.tile([128, NQP], F16, name=f"xpad{cc}", tag=f"xpad{cc}")
            nc.vector.memset(t[:], 0.0)
            dst = t[:, 0:NQ].rearrange("p (h w) -> p h w", w=W2)[:, P:P + H, P:P + W]
            nc.gpsimd.dma_start(out=dst, in_=x_in.ap()[cc * 128:(cc + 1) * 128])
            xpad.append(t)

        # ------------- offset conv: om [27, 2048] -------------
        om_sb = ph1.tile([27, S], F32)
        omT = ph1.tile([128, 16, 32], F32)
        with tc.tile_pool(name="omps", bufs=1, space="PSUM") as omps:
            om_ps = omps.tile([27, S], F32, name="om_ps", tag="om_ps")
            for bk in range(4):           # 4 banks of 512 (8 rows x 64)
                for cc in range(CC):
                    for t9 in range(K):
                        ty, tx = t9 // 3, t9 % 3
                        rhs = xom[cc][:].rearrange("p (h w) -> p h w", w=W2)[
                            :, bk * 8 + ty: bk * 8 + ty + 8,
                            P - 1 + tx: P - 1 + tx + W]
                        nc.tensor.matmul(om_ps[:, bk * 512:(bk + 1) * 512],
                                         woff16[cc][:, t9 * 27:(t9 + 1) * 27], rhs,
                                         start=(cc == 0 and t9 == 0),
                                         stop=(cc == CC - 1 and t9 == K - 1))
            nc.scalar.activation(om_sb[:], om_ps[:], AF.Identity,
                                 bias=boff_t[:, 0:1])

            # ------------- omT [128, 16, 32] -------------
            omT_ps = omps.tile([128, 512], F32, name="omT_ps", tag="omT_ps")
            nc.vector.memset(omT_ps[:], 0.0)
            for ch in range(16):
                nc.tensor.transpose(omT_ps[:, ch * 32: ch * 32 + 27],
                                    om_sb[:, ch * 128:(ch + 1) * 128],
                                    id32[0:27, 0:27])
            nc.vector.tensor_copy(omT[:],
                                  omT_ps[:].rearrange("p (a b) -> p a b", a=16))

        # ------------- sample math -------------
        ppx = ph1.tile([128, 16, 32], F32)
        nc.vector.tensor_tensor(out=ppx[:], in0=omT[:], in1=base_t[:], op=ALU.add)
        ii = ph1.tile([128, 16, 18], I32)
        nc.vector.tensor_copy(ii[:], ppx[:, :, 0:18])
        ff = ph1.tile([128, 16, 18], F32)
        nc.vector.tensor_copy(ff[:], ii[:])
        gtt = ph1.tile([128, 16, 18], F32)
        nc.vector.tensor_tensor(out=gtt[:], in0=ff[:], in1=ppx[:, :, 0:18], op=ALU.is_gt)
        flo = ph1.tile([128, 16, 18], F32)
        nc.vector.tensor_tensor(out=flo[:], in0=ff[:], in1=gtt[:], op=ALU.subtract)
        lf = ph1.tile([128, 16, 18], F32)
        nc.vector.tensor_tensor(out=lf[:], in0=ppx[:, :, 0:18], in1=flo[:], op=ALU.subtract)
        floc = ph1.tile([128, 16, 18], F32)
        nc.vector.tensor_scalar(floc[:], flo[:], 0.0, float(W2 - 2), ALU.max, ALU.min)
        msk = ph1.tile([128, 16, 9], F32)
        nc.scalar.activation(msk[:], omT[:, :, 18:27], AF.Sigmoid)
        ol = ph1.tile([128, 16, 18], F32)
        nc.vector.tensor_scalar(ol[:], lf[:], -1.0, 1.0, ALU.mult, ALU.add)
        # corner weights (with mask folded): [128, 16, 9] each; r order must
        # match the quad token layout: (0,0), (1,0), (0,1), (1,1)
        wr4 = []
        for r, (ya, xa) in enumerate([(0, 0), (1, 0), (0, 1), (1, 1)]):
            yw = ol if ya == 0 else lf     # (1-ly) or ly
            xw = ol if xa == 0 else lf
            wtile = pers.tile([128, 16, 9], F32, name=f"wr4_{r}", tag=f"wr4_{r}")
            nc.vector.tensor_tensor(out=wtile[:], in0=yw[:, :, 0:9],
                                    in1=xw[:, :, 9:18], op=ALU.mult)
            nc.vector.tensor_tensor(out=wtile[:], in0=wtile[:], in1=msk[:], op=ALU.mult)
            wr4.append(wtile)
        # quad token index q = (y0c+1)*W2 + x0c  (front pad of W2 rows)
        qf = ph1.tile([128, 16, 9], F32)
        nc.vector.tensor_scalar(qf[:], floc[:, :, 0:9], float(W2), float(W2),
                                ALU.mult, ALU.add)
        nc.vector.tensor_tensor(out=qf[:], in0=qf[:], in1=floc[:, :, 9:18], op=ALU.add)

        # ------------- gather indices: wrap to [16, 64] + replicate x8 ----
        # wrapped format: token t -> partition t%16, col t//16 with
        # t = chp*128 + s128; sample s128 sits at free pos q*8+jj where
        # s128 = jj*16+q  (so a plain transpose + free-dim permute works).
        idxT16 = ph1.tile([16, K * 128], I16)
        wrapped = []
        with tc.tile_pool(name="idxps", bufs=3, space="PSUM") as idxps:
            for k in range(K):
                tps = idxps.tile([16, 128], F32, name=f"tps{k}", tag="tps")
                nc.tensor.transpose(tps[:], qf[:, :, k:k + 1], id32[:])
                dst = idxT16[:, k * 128:(k + 1) * 128].rearrange(
                    "p (q jj) -> p q jj", q=16)
                src = tps[:].rearrange("p (jj q) -> p q jj", jj=8)
                nc.vector.tensor_copy(dst, src)
            for k in range(K):
                for sblk in range(NSB):
                    src = idxT16[sblk * 8:(sblk + 1) * 8,
                                 k * 128:(k + 1) * 128].rearrange(
                        "p (q jj) -> p q jj", q=16)
                    dstd = idxs_t[k, sblk].rearrange(
                        "q (chp jj) -> q chp jj", chp=8).transpose([1, 0, 2])
                    nc.sync.dma_start(out=dstd, in_=src)
            for sblk in range(NSB):
                wtb = pers.tile([128, K * 64], I16, name=f"wrp{sblk}",
                                tag=f"wrp{sblk}")
                for g in range(8):
                    nc.sync.dma_start(
                        out=wtb[g * 16:(g + 1) * 16, :].rearrange(
                            "q (k c) -> q k c", k=K),
                        in_=idxs_t[:, sblk].transpose([1, 0, 2]))
                wrapped.append(wtb)

        # ------------- XT-quad build -------------
        # quad row r = [XT[r - 74] | XT[r]]; write each transposed XT chunk
        # twice: first halves at rows q+74, second halves at rows q.
        with tc.tile_pool(name="xtp", bufs=2) as xtp, \
             tc.tile_pool(name="xtps", bufs=2, space="PSUM") as xtps:
            for grp in range(11):          # 4 q-chunks per group, 43 chunks
                qcs = list(range(grp * 4, min(grp * 4 + 4, 43)))
                nqc = len(qcs)
                pt = xtps.tile([128, 1024], F16, name=f"xt_ps{grp}", tag="xt_ps")
                for i, qc in enumerate(qcs):
                    for cc in range(CC):
                        nc.tensor.transpose(
                            pt[:, i * 256 + cc * 128: i * 256 + (cc + 1) * 128],
                            xpad[cc][:, qc * 128:(qc + 1) * 128], id16[:])
                st = xtp.tile([128, 1024], F16, name=f"xt_sb{grp}", tag="xt_sb")
                nc.scalar.activation(st[:, 0:nqc * 256], pt[:, 0:nqc * 256], AF.Copy)
                src = st[:, 0:nqc * 256].rearrange("p (qc c) -> p qc c", c=256)
                d1 = quad_t[W2 + grp * 512: W2 + grp * 512 + nqc * 128,
                            0:256].rearrange("(qc p) c -> p qc c", p=128)
                nc.sync.dma_start(out=d1, in_=src)
                d2 = quad_t[grp * 512: grp * 512 + nqc * 128,
                            256:512].rearrange("(qc p) c -> p qc c", p=128)
                nc.sync.dma_start(out=d2, in_=src)

        # gather source AP: overlapping rows (stride 512 elems, len 1024)
        quad_g = quad_t[:, :].copy()
        quad_g.ap[0] = [512, NTOK - 1]
        quad_g.ap[1] = [1, 1024]

        # ------------- main loop -------------
        ph1_cm.__exit__(None, None, None)

        with tc.tile_pool(name="mg", bufs=4) as mg, \
             tc.tile_pool(name="mv", bufs=2) as mv, \
             tc.tile_pool(name="msc", bufs=2) as msc, \
             tc.tile_pool(name="mvs", bufs=2) as mvs, \
             tc.tile_pool(name="mps", bufs=2, space="PSUM") as mps, \
             tc.tile_pool(name="accp", bufs=1, space="PSUM") as accp, \
             tc.tile_pool(name="outp", bufs=2) as outp:
            for sblk in range(NSB):
                accs = {}
                for occ in range(OCC):
                    for sc in range(SB // 512):
                        accs[(occ, sc)] = accp.tile(
                            [128, 512], F32, name=f"acc{sblk}_{occ}_{sc}",
                            tag=f"acc{occ}_{sc}")
                for k in range(K):
                    gt = mg.tile([128, CHB, 1024], F16, name=f"g{sblk}_{k}",
                                 tag="gt")
                    nc.gpsimd.dma_gather(gt[:], quad_g,
                                         wrapped[sblk][:, k * 64:(k + 1) * 64],
                                         NI, NI, 1024, elem_step=512,
                                         single_packet=False)
                    vt = mv.tile([128, CHB * 256], F16, name=f"v{sblk}_{k}",
                                 tag="vt")
                    # Scalar engine pre-scales corners 2,3 (per-partition
                    # scale via activation); DVE does ts + stt + 2 TT adds.
                    s23 = msc.tile([128, CHB, 2, 256], F16,
                                   name=f"s23_{sblk}_{k}", tag="s23")
                    for chp in range(CHB):
                        ch = sblk * CHB + chp
                        for r in (2, 3):
                            nc.scalar.activation(
                                s23[:, chp, r - 2, :],
                                gt[:, chp, r * 256:(r + 1) * 256],
                                AF.Copy, scale=wr4[r][:, ch, k:k + 1])
                    for chp in range(CHB):
                        ch = sblk * CHB + chp
                        vts = vt[:, chp * 256:(chp + 1) * 256]
                        nc.vector.tensor_scalar(
                            vts, gt[:, chp, 0:256],
                            wr4[0][:, ch, k:k + 1], None, ALU.mult)
                        nc.vector.scalar_tensor_tensor(
                            out=vts, in0=gt[:, chp, 256:512],
                            scalar=wr4[1][:, ch, k:k + 1], in1=vts,
                            op0=ALU.mult, op1=ALU.add)
                        for r in (2, 3):
                            nc.vector.tensor_tensor(
                                out=vts, in0=s23[:, chp, r - 2, :],
                                in1=vts, op=ALU.add)
                    # transpose V^T -> V [c, s]
                    vps = [mps.tile([128, 1024], F16, name=f"vps{sblk}_{k}_{cc}",
                                    tag=f"vps{cc}") for cc in range(CC)]
                    for chp in range(CHB):
                        for cc in range(CC):
                            nc.tensor.transpose(
                                vps[cc][:, chp * 128:(chp + 1) * 128],
                                vt[:, chp * 256 + cc * 128: chp * 256 + (cc + 1) * 128],
                                id16[:])
                    vsbk = []
                    for cc in range(CC):
                        t = mvs.tile([128, 1024], F16, name=f"vsb{sblk}_{k}_{cc}",
                                     tag=f"vsb{k}_{cc}")
                        nc.scalar.activation(t[:], vps[cc][:], AF.Copy)
                        vsbk.append(t)
                    # incremental weight-stationary matmuls for this tap
                    for occ in range(OCC):
                        for sc in range(SB // 512):
                            for cc in range(CC):
                                nc.tensor.matmul(
                                    accs[(occ, sc)][:],
                                    wmain[cc][:, k * O + occ * 128: k * O + occ * 128 + 128],
                                    vsbk[cc][:, sc * 512:(sc + 1) * 512],
                                    start=(k == 0 and cc == 0),
                                    stop=(k == K - 1 and cc == CC - 1),
                                    skip_group_check=True)
                for occ in range(OCC):
                    for sc in range(SB // 512):
                        osb = outp.tile([128, 512], F32,
                                        name=f"osb{sblk}_{occ}_{sc}", tag="osb")
                        nc.scalar.activation(osb[:], accs[(occ, sc)][:], AF.Relu,
                                             bias=bprT[:, occ:occ + 1])
                        nc.sync.dma_start(
                            out=out_d.ap()[occ * 128:(occ + 1) * 128,
                                           sblk * SB + sc * 512:
                                           sblk * SB + (sc + 1) * 512],
                            in_=osb[:])


# ===================== host side =====================

def _host_prep(inputs):
    """Per-core input maps: layout + BN/bias folding (no x-dependent work)."""
    x = np.ascontiguousarray(inputs["x"], dtype=np.float32)
    w_off = np.asarray(inputs["w_off"], np.float32)
    b_off = np.asarray(inputs["b_off"], np.float32)
    weight = np.asarray(inputs["weight"], np.float32)
    bias = np.asarray(inputs["bias"], np.float32)
    gamma = np.asarray(inputs["gamma"], np.float32)
    beta = np.asarray(inputs["beta"], np.float32)
    run_mean = np.asarray(inputs["run_mean"], np.float32)
    run_var = np.asarray(inputs["run_var"], np.float32)

    # BN fold: W' = W * sfac[o];  b' = sfac*(bias - mean) + beta
    sfac = gamma / np.sqrt(run_var + EPS)
    bpr = sfac * (bias - run_mean) + beta
    wsc = weight.reshape(O, C, K) * sfac[:, None, None]
    # [O, C, K] -> [C, K, O] -> [CC, 128, K*O], fp16
    wt = wsc.transpose(1, 2, 0).reshape(CC, 128, K * O).astype(np.float16)
    wt = np.ascontiguousarray(wt)
    wofft = w_off.reshape(27, C, K).transpose(1, 2, 0).reshape(
        CC, 128, K * 27).astype(np.float16)
    wofft = np.ascontiguousarray(wofft)
    bprT = np.ascontiguousarray(bpr.reshape(OCC, 128).T.astype(np.float32))
    id32 = np.eye(128, dtype=np.float32)
    id16 = np.eye(128, dtype=np.float16)
    boff = b_off.reshape(27, 1).astype(np.float32)

    in_maps = []
    for core in range(N_CORES):
        b, half = core // 2, core % 2
        h0 = half * HH
        # halo rows [h0-1, h0+33) with zero pad at the image boundary
        halo = np.zeros((C, 34, W), np.float32)
        lo, hi = h0 - 1, h0 + 33
        slo, shi = max(lo, 0), min(hi, H)
        halo[:, slo - lo: slo - lo + (shi - slo)] = x[b, :, slo:shi]
        # baseC [128, 16, 32]: cols 0-8 pyP base, 9-17 pxP base, rest 0
        basec = np.zeros((128, 16, 32), np.float32)
        pp_ = np.arange(128)
        for ch in range(16):
            s_ = ch * 128 + pp_
            hloc = h0 + s_ // W
            wloc = s_ % W
            for k in range(K):
                basec[:, ch, k] = hloc + (k // 3) - 1 + P
                basec[:, ch, 9 + k] = wloc + (k % 3) - 1 + P
        in_maps.append({
            "x_b": np.ascontiguousarray(x[b]),
            "xhalo": halo,
            "wmain16": wt,
            "woff16": wofft,
            "b_off": boff,
            "bprT": bprT,
            "baseC": basec.reshape(128, 16 * 32),
            "ident32": id32,
            "ident16": id16,
        })
    return in_maps


def _get_nc():
    if "nc" not in _NC_CACHE:
        _NC_CACHE["nc"] = build_nc()
    return _NC_CACHE["nc"]


def kernel(**inputs):
    nc = _get_nc()
    in_maps = _host_prep(inputs)
    res = bass_utils.run_bass_kernel_spmd(nc, in_maps, core_ids=list(range(N_CORES)))
    out = np.zeros((B, O, H, W), np.float32)
    for core in range(N_CORES):
        b, half = core // 2, core % 2
        out[b, :, half * HH:(half + 1) * HH, :] = (
            res.results[core]["out_c"].reshape(O, HH, W))
    return out


# revision 12
# speedup vs baseline: 2.5041x; 1.0064x over previous
"""Trainium2 Bass kernel: modulated deformable conv 3x3 (DCNv2) + BN(eval)
+ ReLU.  B=4, C=O=256, H=W=64, distributed over 8 NeuronCores.

Sharding: core i -> batch b = i//2, image row-half = i%2 (32 rows). Each core
computes out[b, :, h0:h0+32, :] fully.

v2 design (quad-token gather):
  - xpad [C,74*74] fp16 zero-padded image (pad P=5) via casting SWDGE DMA;
    xom [C,34*74] halo rows for the offset conv (host-sliced per core).
  - offset conv om[27,2048] (9 shifted matmuls x 2 cc, 4 PSUM banks).
  - omT -> bilinear corner weights wr4 (kept as per-partition scalars) and
    ONE int16 token index per (sample, tap): q = (y0+1)*74 + x0 (clamped to
    the zero pad, so out-of-image corners contribute 0).
  - XT-quad DRAM scratch [NTOK,512] fp16: row r = [X^T[r-74] | X^T[r]].  A
    single 2048B descriptor starting at row r covers rows r,r+1 = all four
    bilinear corners of a sample: 1 descriptor per (sample, tap) instead of
    4, quartering SWDGE descriptor-generation time (the v1 bottleneck).
  - per (sblk, k): one dma_gather (NI=1024, elem 2048B, elem_step 1024B) ->
    gt [128s, 8chp, 1024]; DVE combines the 4 corners (tensor_scalar + 3
    fused scalar_tensor_tensor with per-partition weights); PE transposes
    V^T -> V[c,s]; weight-stationary matmuls accumulate out^T[o,s] in PSUM
    over (k, cc); Scalar engine applies folded BN bias + ReLU from PSUM;
    DMA out^T[o,s] fp32.
BN + conv bias are folded on host: W' = W * (gamma*rsqrt(var+eps)) ;
b' = s*(bias-mean)+beta, applied as per-partition activation bias.
"""

import numpy as np

import concourse.bass as bass
import concourse.bacc as bacc
import concourse.mybir as mybir
import concourse.tile as tile
from concourse import bass_utils, library_config

F32 = mybir.dt.float32
F16 = mybir.dt.float16
I16 = mybir.dt.int16
I32 = mybir.dt.int32
AF = mybir.ActivationFunctionType
ALU = mybir.AluOpType

B, C, O, H, W = 4, 256, 256, 64, 64
K = 9
P = 5
W2 = H + 2 * P            # 74
NQ = W2 * W2              # 5476
NQP = 5504                # 43*128 (transposed q count, padded)
NTOK = 74 + NQP           # 5578 quad rows (front pad of 74)
HH = 32                   # rows per core
S = HH * W                # 2048 samples per core
CC = C // 128             # 2
OCC = O // 128            # 2
NSB = 2                   # sample blocks
SB = S // NSB             # 1024 samples / block
CHB = 8                   # 128-sample chunks per block
NI = SB                   # gather indices per call (1 per sample)
EPS = 1e-5
N_CORES = 8

_NC_CACHE = {}


def build_nc():
    nc = bacc.Bacc("TRN2", target_bir_lowering=False, debug=False,
                   num_devices=N_CORES)

    x_in = nc.dram_tensor("x_b", [C, H, W], F32, kind="ExternalInput")
    xhalo = nc.dram_tensor("xhalo", [C, 34, W], F32, kind="ExternalInput")
    wmain_in = nc.dram_tensor("wmain16", [CC, 128, K * O], F16, kind="ExternalInput")
    woff_in = nc.dram_tensor("woff16", [CC, 128, K * 27], F16, kind="ExternalInput")
    b_off_in = nc.dram_tensor("b_off", [27, 1], F32, kind="ExternalInput")
    bprT_in = nc.dram_tensor("bprT", [128, OCC], F32, kind="ExternalInput")
    baseC = nc.dram_tensor("baseC", [128, 16 * 32], F32, kind="ExternalInput")
    ident32 = nc.dram_tensor("ident32", [128, 128], F32, kind="ExternalInput")
    ident16 = nc.dram_tensor("ident16", [128, 128], F16, kind="ExternalInput")

    out_d = nc.dram_tensor("out_c", [O, S], F32, kind="ExternalOutput")

    with tile.TileContext(nc) as tc:
        _build(nc, tc, x_in, xhalo, wmain_in, woff_in, b_off_in, bprT_in,
               baseC, ident32, ident16, out_d)
    nc.compile()
    return nc


def _build(nc, tc, x_in, xhalo, wmain_in, woff_in, b_off_in, bprT_in,
           baseC, ident32, ident16, out_d):
    from contextlib import ExitStack

    with ExitStack() as top:
        pers = top.enter_context(tc.tile_pool(name="pers", bufs=1))
        dram = top.enter_context(tc.tile_pool(name="dram", bufs=1, space="DRAM"))
        quad_t = dram.tile([NTOK, 512], F16, name="quad_scr", tag="quad")
        idxs_t = dram.tile([K, NSB, 16, 64], I16, name="idx_scr", tag="idxs")
        ph1_cm = tc.tile_pool(name="ph1", bufs=1)
        ph1 = ph1_cm.__enter__()

        # ------------- constants -------------
        id32 = pers.tile([128, 128], F32)
        nc.sync.dma_start(out=id32[:], in_=ident32.ap())
        id16 = pers.tile([128, 128], F16)
        nc.sync.dma_start(out=id16[:], in_=ident16.ap())
        base_t = ph1.tile([128, 16, 32], F32)
        nc.sync.dma_start(out=base_t[:], in_=baseC.ap().rearrange("p (a b) -> p a b", a=16))
        boff_t = ph1.tile([27, 1], F32)
        nc.sync.dma_start(out=boff_t[:], in_=b_off_in.ap())
        bprT = pers.tile([128, OCC], F32)
        nc.sync.dma_start(out=bprT[:], in_=bprT_in.ap())
        wmain = []
        for cc in range(CC):
            t = pers.tile([128, K * O], F16, name=f"wmain{cc}", tag=f"wmain{cc}")
            nc.sync.dma_start(out=t[:], in_=wmain_in.ap()[cc])
            wmain.append(t)
        woff16 = []
        for cc in range(CC):
            t = ph1.tile([128, K * 27], F16, name=f"woff{cc}", tag=f"woff{cc}")
            nc.sync.dma_start(out=t[:], in_=woff_in.ap()[cc])
            woff16.append(t)

        # ------------- xom (halo, fp16) + xpad (full, fp16) -------------
        xom = []
        for cc in range(CC):
            t = ph1.tile([128, 34 * W2], F16, name=f"xom{cc}", tag=f"xom{cc}")
            nc.vector.memset(t[:], 0.0)
            dst = t[:].rearrange("p (h w) -> p h w", w=W2)[:, :, P:P + W]
            nc.gpsimd.dma_start(out=dst, in_=xhalo.ap()[cc * 128:(cc + 1) * 128])
            xom.append(t)
        xpad = []
        for cc in range(CC):
            t = ph1.tile([128, NQP], F16, name=f"xpad{cc}", tag=f"xpad{cc}")
            nc.vector.memset(t[:], 0.0)
            dst = t[:, 0:NQ].rearrange("p (h w) -> p h w", w=W2)[:, P:P + H, P:P + W]
            nc.gpsimd.dma_start(out=dst, in_=x_in.ap()[cc * 128:(cc + 1) * 128])
            xpad.append(t)

        # ------------- offset conv: om [27, 2048] -------------
        om_sb = ph1.tile([27, S], F32)
        omT = ph1.tile([128, 16, 32], F32)
        with tc.tile_pool(name="omps", bufs=1, space="PSUM") as omps:
            om_ps = omps.tile([27, S], F32, name="om_ps", tag="om_ps")
            for bk in range(4):           # 4 banks of 512 (8 rows x 64)
                for cc in range(CC):
                    for t9 in range(K):
                        ty, tx = t9 // 3, t9 % 3
                        rhs = xom[cc][:].rearrange("p (h w) -> p h w", w=W2)[
                            :, bk * 8 + ty: bk * 8 + ty + 8,
                            P - 1 + tx: P - 1 + tx + W]
                        nc.tensor.matmul(om_ps[:, bk * 512:(bk + 1) * 512],
                                         woff16[cc][:, t9 * 27:(t9 + 1) * 27], rhs,
                                         start=(cc == 0 and t9 == 0),
                                         stop=(cc == CC - 1 and t9 == K - 1))
            nc.scalar.activation(om_sb[:], om_ps[:], AF.Identity,
                                 bias=boff_t[:, 0:1])

            # ------------- omT [128, 16, 32] -------------
            omT_ps = omps.tile([128, 512], F32, name="omT_ps", tag="omT_ps")
            nc.vector.memset(omT_ps[:], 0.0)
            for ch in range(16):
                nc.tensor.transpose(omT_ps[:, ch * 32: ch * 32 + 27],
                                    om_sb[:, ch * 128:(ch + 1) * 128],
                                    id32[0:27, 0:27])
            nc.vector.tensor_copy(omT[:],
                                  omT_ps[:].rearrange("p (a b) -> p a b", a=16))

        # ------------- sample math -------------
        ppx = ph1.tile([128, 16, 32], F32)
        nc.vector.tensor_tensor(out=ppx[:], in0=omT[:], in1=base_t[:], op=ALU.add)
        ii = ph1.tile([128, 16, 18], I32)
        nc.vector.tensor_copy(ii[:], ppx[:, :, 0:18])
        ff = ph1.tile([128, 16, 18], F32)
        nc.vector.tensor_copy(ff[:], ii[:])
        gtt = ph1.tile([128, 16, 18], F32)
        nc.vector.tensor_tensor(out=gtt[:], in0=ff[:], in1=ppx[:, :, 0:18], op=ALU.is_gt)
        flo = ph1.tile([128, 16, 18], F32)
        nc.vector.tensor_tensor(out=flo[:], in0=ff[:], in1=gtt[:], op=ALU.subtract)
        lf = ph1.tile([128, 16, 18], F32)
        nc.vector.tensor_tensor(out=lf[:], in0=ppx[:, :, 0:18], in1=flo[:], op=ALU.subtract)
        floc = ph1.tile([128, 16, 18], F32)
        nc.vector.tensor_scalar(floc[:], flo[:], 0.0, float(W2 - 2), ALU.max, ALU.min)
        msk = ph1.tile([128, 16, 9], F32)
        nc.scalar.activation(msk[:], omT[:, :, 18:27], AF.Sigmoid)
        ol = ph1.tile([128, 16, 18], F32)
        nc.vector.tensor_scalar(ol[:], lf[:], -1.0, 1.0, ALU.mult, ALU.add)
        # corner weights (with mask folded): [128, 16, 9] each; r order must
        # match the quad token layout: (0,0), (1,0), (0,1), (1,1)
        wr4 = []
        for r, (ya, xa) in enumerate([(0, 0), (1, 0), (0, 1), (1, 1)]):
            yw = ol if ya == 0 else lf     # (1-ly) or ly
            xw = ol if xa == 0 else lf
            wtile = pers.tile([128, 16, 9], F32, name=f"wr4_{r}", tag=f"wr4_{r}")
            nc.vector.tensor_tensor(out=wtile[:], in0=yw[:, :, 0:9],
                                    in1=xw[:, :, 9:18], op=ALU.mult)
            nc.vector.tensor_tensor(out=wtile[:], in0=wtile[:], in1=msk[:], op=ALU.mult)
            wr4.append(wtile)
        # quad token index q = (y0c+1)*W2 + x0c  (front pad of W2 rows)
        qf = ph1.tile([128, 16, 9], F32)
        nc.vector.tensor_scalar(qf[:], floc[:, :, 0:9], float(W2), float(W2),
                                ALU.mult, ALU.add)
        nc.vector.tensor_tensor(out=qf[:], in0=qf[:], in1=floc[:, :, 9:18], op=ALU.add)

        # ------------- gather indices: wrap to [16, 64] + replicate x8 ----
        # wrapped format: token t -> partition t%16, col t//16 with
        # t = chp*128 + s128; sample s128 sits at free pos q*8+jj where
        # s128 = jj*16+q  (so a plain transpose + free-dim permute works).
        idxT16 = ph1.tile([16, K * 128], I16)
        wrapped = []
        with tc.tile_pool(name="idxps", bufs=3, space="PSUM") as idxps:
            for k in range(K):
                tps = idxps.tile([16, 128], F32, name=f"tps{k}", tag="tps")
                nc.tensor.transpose(tps[:], qf[:, :, k:k + 1], id32[:])
                dst = idxT16[:, k * 128:(k + 1) * 128].rearrange(
                    "p (q jj) -> p q jj", q=16)
                src = tps[:].rearrange("p (jj q) -> p q jj", jj=8)
                nc.vector.tensor_copy(dst, src)
            for k in range(K):
                for sblk in range(NSB):
                    src = idxT16[sblk * 8:(sblk + 1) * 8,
                                 k * 128:(k + 1) * 128].rearrange(
                        "p (q jj) -> p q jj", q=16)
                    dstd = idxs_t[k, sblk].rearrange(
                        "q (chp jj) -> q chp jj", chp=8).transpose([1, 0, 2])
                    nc.sync.dma_start(out=dstd, in_=src)
            for sblk in range(NSB):
                wtb = pers.tile([128, K * 64], I16, name=f"wrp{sblk}",
                                tag=f"wrp{sblk}")
                for g in range(8):
                    nc.sync.dma_start(
                        out=wtb[g * 16:(g + 1) * 16, :].rearrange(
                            "q (k c) -> q k c", k=K),
                        in_=idxs_t[:, sblk].transpose([1, 0, 2]))
                wrapped.append(wtb)

        # ------------- XT-quad build -------------
        # quad row r = [XT[r - 74] | XT[r]]; write each transposed XT chunk
        # twice: first halves at rows q+74, second halves at rows q.
        with tc.tile_pool(name="xtp", bufs=2) as xtp, \
             tc.tile_pool(name="xtps", bufs=2, space="PSUM") as xtps:
            for grp in range(11):          # 4 q-chunks per group, 43 chunks
                qcs = list(range(grp * 4, min(grp * 4 + 4, 43)))
                nqc = len(qcs)
                pt = xtps.tile([128, 1024], F16, name=f"xt_ps{grp}", tag="xt_ps")
                for i, qc in enumerate(qcs):
                    for cc in range(CC):
                        nc.tensor.transpose(
                            pt[:, i * 256 + cc * 128: i * 256 + (cc + 1) * 128],
                            xpad[cc][:, qc * 128:(qc + 1) * 128], id16[:])
                st = xtp.tile([128, 1024], F16, name=f"xt_sb{grp}", tag="xt_sb")
                nc.scalar.activation(st[:, 0:nqc * 256], pt[:, 0:nqc * 256], AF.Copy)
                src = st[:, 0:nqc * 256].rearrange("p (qc c) -> p qc c", c=256)
                d1 = quad_t[W2 + grp * 512: W2 + grp * 512 + nqc * 128,
                            0:256].rearrange("(qc p) c -> p qc c", p=128)
                nc.sync.dma_start(out=d1, in_=src)
                d2 = quad_t[grp * 512: grp * 512 + nqc * 128,
                            256:512].rearrange("(qc p) c -> p qc c", p=128)
                nc.sync.dma_start(out=d2, in_=src)

        # gather source AP: overlapping rows (stride 512 elems, len 1024)
        quad_g = quad_t[:, :].copy()
        quad_g.ap[0] = [512, NTOK - 1]
        quad_g.ap[1] = [1, 1024]

        # ------------- main loop -------------
        ph1_cm.__exit__(None, None, None)

        with tc.tile_pool(name="mg", bufs=4) as mg, \
             tc.tile_pool(name="mv", bufs=2) as mv, \
             tc.tile_pool(name="msc", bufs=2) as msc, \
             tc.tile_pool(name="mvs", bufs=2) as mvs, \
             tc.tile_pool(name="mps", bufs=2, space="PSUM") as mps, \
             tc.tile_pool(name="accp", bufs=1, space="PSUM") as accp, \
             tc.tile_pool(name="outp", bufs=2) as outp:
            for sblk in range(NSB):
                accs = {}
                for occ in range(OCC):
                    for sc in range(SB // 512):
                        accs[(occ, sc)] = accp.tile(
                            [128, 512], F32, name=f"acc{sblk}_{occ}_{sc}",
                            tag=f"acc{occ}_{sc}")
                # one-tap skew: vsb copies + matmuls for tap k-1 are emitted
                # after tap k's Scalar scaling acts, so the in-order Scalar
                # queue never blocks the next tap's combine inputs.
                vps_pend = {}

                def drain_tap(kp):
                    vps_k = vps_pend.pop(kp)
                    for cc in range(CC):
                        t = mvs.tile([128, 1024], F16,
                                     name=f"vsb{sblk}_{kp}_{cc}",
                                     tag=f"vsb{kp}_{cc}")
                        nc.scalar.activation(t[:], vps_k[cc][:], AF.Copy)
                        for occ in range(OCC):
                            for sc in range(SB // 512):
                                nc.tensor.matmul(
                                    accs[(occ, sc)][:],
                                    wmain[cc][:, kp * O + occ * 128:
                                               kp * O + occ * 128 + 128],
                                    t[:, sc * 512:(sc + 1) * 512],
                                    start=(kp == 0 and cc == 0),
                                    stop=(kp == K - 1 and cc == CC - 1),
                                    skip_group_check=True)

                for k in range(K):
                    gt = mg.tile([128, CHB, 1024], F16, name=f"g{sblk}_{k}",
                                 tag="gt")
                    nc.gpsimd.dma_gather(gt[:], quad_g,
                                         wrapped[sblk][:, k * 64:(k + 1) * 64],
                                         NI, NI, 1024, elem_step=512,
                                         single_packet=False)
                    vt = mv.tile([128, CHB * 256], F16, name=f"v{sblk}_{k}",
                                 tag="vt")
                    # Scalar engine pre-scales corners 2,3 (per-partition
                    # scale via activation); DVE does ts + stt + 2 TT adds.
                    s23 = msc.tile([128, CHB, 2, 256], F16,
                                   name=f"s23_{sblk}_{k}", tag="s23")
                    for chp in range(CHB):
                        ch = sblk * CHB + chp
                        for r in (2, 3):
                            nc.scalar.activation(
                                s23[:, chp, r - 2, :],
                                gt[:, chp, r * 256:(r + 1) * 256],
                                AF.Copy, scale=wr4[r][:, ch, k:k + 1])
                    if k > 0:
                        drain_tap(k - 1)
                    vps = [mps.tile([128, 1024], F16, name=f"vps{sblk}_{k}_{cc}",
                                    tag=f"vps{cc}") for cc in range(CC)]
                    for chp in range(CHB):
                        ch = sblk * CHB + chp
                        vts = vt[:, chp * 256:(chp + 1) * 256]
                        nc.vector.tensor_scalar(
                            vts, gt[:, chp, 0:256],
                            wr4[0][:, ch, k:k + 1], None, ALU.mult)
                        nc.vector.scalar_tensor_tensor(
                            out=vts, in0=gt[:, chp, 256:512],
                            scalar=wr4[1][:, ch, k:k + 1], in1=vts,
                            op0=ALU.mult, op1=ALU.add)
                        for r in (2, 3):
                            nc.vector.tensor_tensor(
                                out=vts, in0=s23[:, chp, r - 2, :],
                                in1=vts, op=ALU.add)
                        # transpose this chunk V^T -> V [c, s]
                        for cc in range(CC):
                            nc.tensor.transpose(
                                vps[cc][:, chp * 128:(chp + 1) * 128],
                                vt[:, chp * 256 + cc * 128: chp * 256 + (cc + 1) * 128],
                                id16[:])
                    vps_pend[k] = vps
                drain_tap(K - 1)
                for occ in range(OCC):
                    for sc in range(SB // 512):
                        osb = outp.tile([128, 512], F32,
                                        name=f"osb{sblk}_{occ}_{sc}", tag="osb")
                        nc.scalar.activation(osb[:], accs[(occ, sc)][:], AF.Relu,
                                             bias=bprT[:, occ:occ + 1])
                        nc.sync.dma_start(
                            out=out_d.ap()[occ * 128:(occ + 1) * 128,
                                           sblk * SB + sc * 512:
                                           sblk * SB + (sc + 1) * 512],
                            in_=osb[:])


# ===================== host side =====================

def _host_prep(inputs):
    """Per-core input maps: layout + BN/bias folding (no x-dependent work)."""
    x = np.ascontiguousarray(inputs["x"], dtype=np.float32)
    w_off = np.asarray(inputs["w_off"], np.float32)
    b_off = np.asarray(inputs["b_off"], np.float32)
    weight = np.asarray(inputs["weight"], np.float32)
    bias = np.asarray(inputs["bias"], np.float32)
    gamma = np.asarray(inputs["gamma"], np.float32)
    beta = np.asarray(inputs["beta"], np.float32)
    run_mean = np.asarray(inputs["run_mean"], np.float32)
    run_var = np.asarray(inputs["run_var"], np.float32)

    # BN fold: W' = W * sfac[o];  b' = sfac*(bias - mean) + beta
    sfac = gamma / np.sqrt(run_var + EPS)
    bpr = sfac * (bias - run_mean) + beta
    wsc = weight.reshape(O, C, K) * sfac[:, None, None]
    # [O, C, K] -> [C, K, O] -> [CC, 128, K*O], fp16
    wt = wsc.transpose(1, 2, 0).reshape(CC, 128, K * O).astype(np.float16)
    wt = np.ascontiguousarray(wt)
    wofft = w_off.reshape(27, C, K).transpose(1, 2, 0).reshape(
        CC, 128, K * 27).astype(np.float16)
    wofft = np.ascontiguousarray(wofft)
    bprT = np.ascontiguousarray(bpr.reshape(OCC, 128).T.astype(np.float32))
    id32 = np.eye(128, dtype=np.float32)
    id16 = np.eye(128, dtype=np.float16)
    boff = b_off.reshape(27, 1).astype(np.float32)

    in_maps = []
    for core in range(N_CORES):
        b, half = core // 2, core % 2
        h0 = half * HH
        # halo rows [h0-1, h0+33) with zero pad at the image boundary
        halo = np.zeros((C, 34, W), np.float32)
        lo, hi = h0 - 1, h0 + 33
        slo, shi = max(lo, 0), min(hi, H)
        halo[:, slo - lo: slo - lo + (shi - slo)] = x[b, :, slo:shi]
        # baseC [128, 16, 32]: cols 0-8 pyP base, 9-17 pxP base, rest 0
        basec = np.zeros((128, 16, 32), np.float32)
        pp_ = np.arange(128)
        for ch in range(16):
            s_ = ch * 128 + pp_
            hloc = h0 + s_ // W
            wloc = s_ % W
            for k in range(K):
                basec[:, ch, k] = hloc + (k // 3) - 1 + P
                basec[:, ch, 9 + k] = wloc + (k % 3) - 1 + P
        in_maps.append({
            "x_b": np.ascontiguousarray(x[b]),
            "xhalo": halo,
            "wmain16": wt,
            "woff16": wofft,
            "b_off": boff,
            "bprT": bprT,
            "baseC": basec.reshape(128, 16 * 32),
            "ident32": id32,
            "ident16": id16,
        })
    return in_maps


def _get_nc():
    if "nc" not in _NC_CACHE:
        _NC_CACHE["nc"] = build_nc()
    return _NC_CACHE["nc"]


def kernel(**inputs):
    nc = _get_nc()
    in_maps = _host_prep(inputs)
    res = bass_utils.run_bass_kernel_spmd(nc, in_maps, core_ids=list(range(N_CORES)))
    out = np.zeros((B, O, H, W), np.float32)
    for core in range(N_CORES):
        b, half = core // 2, core % 2
        out[b, :, half * HH:(half + 1) * HH, :] = (
            res.results[core]["out_c"].reshape(O, HH, W))
    return out
